# revision 3
# baseline (speedup 1.0000x reference)
"""Multi-head attention (B=2, S=2048, D=1024, H=16, A=64) on 8 TRN2 NeuronCores.

Sharding: core c = b*4 + g handles batch b and head-group g (4 heads).
Tensor-parallel over heads; the all-reduce over head groups is host-side
during the gather (sum of 4 bf16 partials per batch, f32 accumulate).

Key structure (v2 — exp-stream-centric schedule):
 - The Scalar (ACT) engine's exp stream is the critical path (~9.4M exps/core).
   The schedule starts that stream as early as possible and keeps it gapless:
   scores for one head pair land in two alternating PSUM tiles (sc_e/sc_o)
   that exp drains back-to-back while the PE refills the other; everything
   else (projections, PV, out-proj) is PE filler emitted between score
   matmuls in dependency-safe program order.
 - Masking is folded into the data, not the exp: invalid/padded key columns
   of xkv are zeroed host-side (k=0 -> score 0 -> exp 1 -> v=0 contributes
   nothing) and the softmax denominator comes from a 0/1 mask column
   appended to V, so masked keys count 0 in the denominator too.
 - Scores for the two heads of a pair are emitted interleaved (rows 0:64 via
   PE tile T0, rows 64:128 via T8) so the row-tiled PE can overlap them.
 - ~140 dummy warm-up matmuls at t=0 keep the PE busy while input DMAs land,
   flipping the HAM clock gate to 2.4 GHz before real work starts.
 - Output projection streams per 128-query tile into bf16 DRAM as soon as
   each half's context is normalized; host gather sums partials in f32.
"""

import numpy as np

import concourse.bass as bass
import concourse.bacc as bacc
import concourse.mybir as mybir
import concourse.tile as tile
from concourse.bass_utils import run_bass_kernel_spmd

F32 = mybir.dt.float32
BF16 = mybir.dt.bfloat16
DT_MM = BF16
EXP = mybir.ActivationFunctionType.Exp

B = 2
S = 2048
D = 1024
H = 16
A = 64
HG = 4           # head groups (cores per batch)
HL = H // HG     # heads per core = 4
DSUB = D // 128  # 8
QCH = 512        # matmul qi chunk
HALF = 1024      # qi half processed per quarter


def build_program(C: int) -> bass.Bass:
    """Per-core SPMD program for key-capacity C (multiple of 128)."""
    KT = C // 128
    # kv windows: first is 128 keys so scores can start early
    kwins = [(0, 128)]
    pos = 128
    while pos < C:
        w = min(QCH, C - pos)
        kwins.append((pos, w))
        pos += w
    n_pr_bufs = 24 if KT > 10 else min(44, 4 * KT + 2)

    nc = bacc.Bacc("TRN2", target_bir_lowering=False, name=f"mha2_c{C}")
    xq_d = nc.dram_tensor("xq", [D, S], DT_MM, kind="ExternalInput")
    xkv_d = nc.dram_tensor("xkv", [D, C], DT_MM, kind="ExternalInput")
    wqkv_d = nc.dram_tensor("wqkv", [D, 3 * HL * A], DT_MM, kind="ExternalInput")
    wout_d = nc.dram_tensor("wout", [HL * A, D], DT_MM, kind="ExternalInput")
    mcol_d = nc.dram_tensor("mcol", [C], DT_MM, kind="ExternalInput")
    out_d = nc.dram_tensor("out", [S, D], BF16, kind="ExternalOutput")

    with tile.TileContext(nc) as tc:
        with (
            tc.tile_pool(name="const", bufs=1) as const,
            tc.tile_pool(name="probs", bufs=n_pr_bufs) as probs,
            tc.tile_pool(name="norm", bufs=3) as norm,
            tc.tile_pool(name="dramp", bufs=4, space="DRAM") as dramp,
            tc.tile_pool(name="outp", bufs=3) as outp,
            tc.tile_pool(name="psS", bufs=2, space="PSUM") as psS,
            tc.tile_pool(name="psW", bufs=4, space="PSUM") as psW,
        ):
            # ---- persistent SBUF residents ----
            w_sb = const.tile([128, DSUB, 3 * HL * A], DT_MM)
            wout_sb = const.tile([128, 2, D], DT_MM)
            qT = const.tile([128, 2, S], DT_MM)        # [hp*64+a, h2, qi]
            kT = const.tile([128, 2, C], DT_MM)        # [hp*64+a, h2, ki]
            vx = const.tile([128, KT, HL, A + 1], DT_MM)  # [ki%128, kt, h, a|mask]
            ctxT = const.tile([128, 2, S], DT_MM)      # [hp*64+a, h2, qi]
            warm = const.tile([128, 128], DT_MM)
            xq_t = const.tile([128, DSUB, S], DT_MM)
            xkva = const.tile([128, DSUB, 128], DT_MM)
            xkvb = const.tile([128, DSUB, C - 128], DT_MM)

            nc.vector.memset(warm, 0.0)

            w_r = wqkv_d.ap().rearrange("(o p) j -> p o j", p=128)
            xq_r = xq_d.ap().rearrange("(o p) s -> p o s", p=128)
            xkv_r = xkv_d.ap().rearrange("(o p) s -> p o s", p=128)
            mcol_r = mcol_d.ap().rearrange("(t p) -> p t", p=128)

            # ---- DMA issue order (earliest-needed first) ----
            nc.sync.dma_start(w_sb[:, :, 0:256], w_r[:, :, 0:256])      # wq
            nc.sync.dma_start(xq_t[:, :, 0:512], xq_r[:, :, 0:512])
            nc.sync.dma_start(xq_t[:, :, 512:1024], xq_r[:, :, 512:1024])
            nc.sync.dma_start(w_sb[:, :, 256:512], w_r[:, :, 256:512])  # wk
            nc.sync.dma_start(xkva, xkv_r[:, :, 0:128])
            for base, w in kwins[1:]:
                nc.sync.dma_start(xkvb[:, :, base - 128:base - 128 + w],
                                  xkv_r[:, :, base:base + w])
            nc.sync.dma_start(w_sb[:, :, 512:768], w_r[:, :, 512:768])  # wv
            for h in range(HL):
                nc.sync.dma_start(vx[:, :, h, A], mcol_r)
            nc.sync.dma_start(xq_t[:, :, 1024:1536], xq_r[:, :, 1024:1536])
            nc.sync.dma_start(xq_t[:, :, 1536:2048], xq_r[:, :, 1536:2048])
            nc.sync.dma_start(
                wout_sb, wout_d.ap().rearrange("(s p) d -> p s d", p=128)
            )

            # ---- HAM warm-up: keep PE busy while input DMAs land ----
            wp = psW.tile([128, 512], F32, tag="w", name="warmps")
            for _ in range(140):
                nc.tensor.matmul(wp[:, 0:128], warm, warm, start=True, stop=True)

            # ---- emission helpers (each is one PE "filler unit") ----
            def qproj_ct(c, ct):
                ps = psW.tile([128, 512], F32, tag="w", name=f"qp{c}_{ct}")
                for o in range(DSUB):
                    nc.tensor.matmul(
                        ps,
                        w_sb[:, o, ct * 128:(ct + 1) * 128],
                        xq_t[:, o, c * QCH:(c + 1) * QCH],
                        start=(o == 0),
                        stop=(o == DSUB - 1),
                    )
                nc.vector.tensor_copy(out=qT[:, ct, c * QCH:(c + 1) * QCH],
                                      in_=ps)

            def xkv_ap(base, w):
                if base == 0:
                    assert w <= 128
                    return xkva[:, :, 0:w]
                return xkvb[:, :, base - 128:base - 128 + w]

            def kproj_ct(wi, ct):
                base, w = kwins[wi]
                src = xkv_ap(base, w)
                ps = psW.tile([128, 512], F32, tag="w", name=f"kp{wi}_{ct}")
                for o in range(DSUB):
                    nc.tensor.matmul(
                        ps[:, :w],
                        w_sb[:, o, 256 + ct * 128:256 + (ct + 1) * 128],
                        src[:, o, :],
                        start=(o == 0),
                        stop=(o == DSUB - 1),
                    )
                nc.vector.tensor_copy(out=kT[:, ct, base:base + w],
                                      in_=ps[:, :w])

            def vproj_kt(kt):
                src = xkv_ap(kt * 128, 128)
                ps = psW.tile([128, 512], F32, tag="w", name=f"vp{kt}")
                pv = ps[:, 0:HL * A]
                for o in range(DSUB):
                    nc.tensor.matmul(
                        pv,
                        src[:, o, :],
                        w_sb[:, o, 512:512 + HL * A],
                        start=(o == 0),
                        stop=(o == DSUB - 1),
                    )
                nc.vector.tensor_copy(
                    out=vx[:, kt, :, 0:A],
                    in_=pv.rearrange("p (h a) -> p h a", a=A),
                )

            prs = {}

            def scores_pair(half, h2, kt):
                """Interleaved T0/T8 score matmuls + 2 exps for a head pair."""
                q0 = half * HALF
                sc_e = psS.tile([128, HALF], F32, tag="sc",
                                name=f"sce{half}_{h2}_{kt}")
                sc_o = psS.tile([128, HALF], F32, tag="sc",
                                name=f"sco{half}_{h2}_{kt}")
                for cc in range(2):
                    qs_ = slice(q0 + cc * QCH, q0 + (cc + 1) * QCH)
                    cs = slice(cc * QCH, (cc + 1) * QCH)
                    nc.tensor.matmul(
                        sc_e[:, cs],
                        kT[0:64, h2, kt * 128:(kt + 1) * 128],
                        qT[0:64, h2, qs_],
                        start=True, stop=True,
                    )
                    nc.tensor.matmul(
                        sc_o[:, cs],
                        kT[64:128, h2, kt * 128:(kt + 1) * 128],
                        qT[64:128, h2, qs_],
                        start=True, stop=True,
                    )
                pr_e = probs.tile([128, HALF], DT_MM, tag="pr",
                                  name=f"pre{half}_{h2}_{kt}")
                pr_o = probs.tile([128, HALF], DT_MM, tag="pr",
                                  name=f"pro{half}_{h2}_{kt}")
                nc.scalar.activation(out=pr_e, in_=sc_e, func=EXP)
                nc.scalar.activation(out=pr_o, in_=sc_o, func=EXP)
                prs[(half, 2 * h2, kt)] = pr_e
                prs[(half, 2 * h2 + 1, kt)] = pr_o

            def pv_unit(h, half, cl):
                """PV accumulation for (head, half, chunk-in-half cl).
                Returns a closure finishing the normalize chain."""
                hp, h2 = h % 2, h // 2
                pvt = psW.tile([128, 512], F32, tag="w",
                               name=f"pv{h}_{half}_{cl}")
                pva = pvt[0:A + 1, :]
                for kt in range(KT):
                    nc.tensor.matmul(
                        pva,
                        vx[:, kt, h, :],
                        prs[(half, h, kt)][:, cl * QCH:(cl + 1) * QCH],
                        start=(kt == 0),
                        stop=(kt == KT - 1),
                    )
                # stage 1 of normalize: denom row -> SBUF -> DRAM -> broadcast
                dnr = norm.tile([65, 512], F32, tag="dnr")
                nc.vector.tensor_copy(out=dnr[64:65, :], in_=pvt[A:A + 1, :])
                dnd = dramp.tile([1, 512], F32, tag="dnd")
                nc.gpsimd.dma_start(dnd, dnr[64:65, :])
                rBr = norm.tile([64, 512], F32, tag="rBr")
                dnd_b = bass.AP(
                    tensor=dnd.tensor,
                    offset=dnd.offset,
                    ap=[[0, 64], list(dnd.ap[1])],
                )
                nc.gpsimd.dma_start(rBr, dnd_b)

                cslice = slice(half * HALF + cl * QCH,
                               half * HALF + (cl + 1) * QCH)

                def finish():
                    rB = norm.tile([64, 512], F32, tag="rB")
                    nc.vector.reciprocal_approx_fast(rB, rBr)
                    if hp == 0:
                        nc.vector.tensor_tensor(
                            ctxT[0:64, h2, cslice], pvt[0:A, :], rB,
                            mybir.AluOpType.mult,
                        )
                    else:
                        stg = norm.tile([64, 512], DT_MM, tag="stg")
                        nc.vector.tensor_tensor(
                            stg, pvt[0:A, :], rB, mybir.AluOpType.mult
                        )
                        nc.gpsimd.dma_start(ctxT[64:128, h2, cslice], stg)

                return finish

            def outproj_st(st):
                """Output projection + bf16 drain + DMA for query tile st."""
                ot = outp.tile([128, D], BF16, tag="ot", name=f"ot{st}")
                for dc in range(2):
                    po = psW.tile([128, 512], F32, tag="w", name=f"po{st}_{dc}")
                    for s2 in range(2):
                        nc.tensor.matmul(
                            po,
                            ctxT[:, s2, st * 128:(st + 1) * 128],
                            wout_sb[:, s2, dc * 512:(dc + 1) * 512],
                            start=(s2 == 0),
                            stop=(s2 == 1),
                        )
                    nc.vector.tensor_copy(
                        out=ot[:, dc * 512:(dc + 1) * 512], in_=po
                    )
                nc.sync.dma_start(out_d.ap()[st * 128:(st + 1) * 128, :], ot)

            # ---- lead-in PE work (minimum needed for first scores) ----
            qproj_ct(0, 0)
            qproj_ct(1, 0)
            kproj_ct(0, 0)

            pending_finish = []

            def run_unit(u):
                # flush one staged normalize-finish before each filler unit so
                # the DVE queue isn't head-of-line blocked on the broadcast DMA
                if pending_finish:
                    pending_finish.pop(0)()
                kind = u[0]
                if kind == "q":
                    qproj_ct(u[1], u[2])
                elif kind == "k":
                    kproj_ct(u[1], u[2])
                elif kind == "v":
                    vproj_kt(u[1])
                elif kind == "pv":
                    pending_finish.append(pv_unit(u[1], u[2], u[3]))
                elif kind == "st":
                    outproj_st(u[1])

            # ---- per-quarter filler slots (emitted AFTER each slot's scores;
            #      every unit must precede, in PE order, the first score matmul
            #      that depends on it) ----
            def build_slots(units, KT):
                slots = [[] for _ in range(KT)]
                for i, u in enumerate(units):
                    slots[min(KT - 1, 1 + i * (KT - 1) // max(1, len(units)))
                          ].append(u)
                return slots

            # quarter (0,0): k-proj windows must be emitted before the first
            # score slot of their keys; q-ct1 and v-proj spread behind
            q00 = [[] for _ in range(KT)]
            for wi in range(1, len(kwins)):
                first_kt = kwins[wi][0] // 128
                q00[max(0, first_kt - 4)] += [("k", wi, 0), ("k", wi, 1)]
            q00[0] += [("k", 0, 1)]
            rest = [("q", 0, 1), ("q", 1, 1)] + [("v", kt) for kt in range(KT)]
            for i, u in enumerate(rest):
                q00[1 + (i % (KT - 1))].append(u)

            quarter_slots = {
                (0, 0): q00,
                (0, 1): build_slots(
                    [("q", 2, 0), ("q", 2, 1), ("q", 3, 0), ("q", 3, 1),
                     ("pv", 0, 0, 0), ("pv", 1, 0, 0),
                     ("pv", 0, 0, 1), ("pv", 1, 0, 1)], KT),
                (1, 0): build_slots(
                    [("pv", 2, 0, 0), ("pv", 3, 0, 0),
                     ("pv", 2, 0, 1), ("pv", 3, 0, 1),
                     ("st", 0), ("st", 1), ("st", 2), ("st", 3),
                     ("st", 4), ("st", 5), ("st", 6), ("st", 7)], KT),
                (1, 1): build_slots(
                    [("pv", 0, 1, 0), ("pv", 1, 1, 0),
                     ("pv", 0, 1, 1), ("pv", 1, 1, 1)], KT),
            }

            for half in range(2):
                for h2 in range(2):
                    slots = quarter_slots[(half, h2)]
                    for kt in range(KT):
                        scores_pair(half, h2, kt)
                        for u in slots[kt]:
                            run_unit(u)

            # ---- tail: last pair's PV + second-half out-proj ----
            for u in [("pv", 2, 1, 0), ("pv", 3, 1, 0),
                      ("st", 8), ("st", 9), ("st", 10), ("st", 11),
                      ("pv", 2, 1, 1), ("pv", 3, 1, 1),
                      ("st", 12), ("st", 13), ("st", 14), ("st", 15)]:
                run_unit(u)
            while pending_finish:
                pending_finish.pop(0)()

    return nc


_PROGRAM_CACHE: dict[int, bass.Bass] = {}


def _get_program(C: int) -> bass.Bass:
    if C not in _PROGRAM_CACHE:
        nc = build_program(C)
        nc.finalize()
        _PROGRAM_CACHE[C] = nc
    return _PROGRAM_CACHE[C]


def _ceil128(n: int) -> int:
    return max(128, (n + 127) // 128 * 128)


def prepare_in_maps(qs, mask, Wqkv, Wout):
    """Shard FULL inputs into 8 per-core input maps. Returns (in_maps, C)."""
    import ml_dtypes

    np_mm = ml_dtypes.bfloat16
    qs = np.ascontiguousarray(qs, dtype=np.float32)
    mask = np.asarray(mask)
    Wqkv = np.ascontiguousarray(Wqkv, dtype=np.float32)
    Wout = np.ascontiguousarray(Wout, dtype=np.float32)

    nvalid = [int(np.count_nonzero(mask[b])) for b in range(B)]
    if min(nvalid) == 0:
        C = S  # degenerate masks: run dense
    else:
        C = min(S, _ceil128(max(nvalid)))
    compact = C < S

    xq, xkv, mc = [], [], []
    for b in range(B):
        xq.append(np.ascontiguousarray(qs[b].T.astype(np_mm)))
        if compact:
            idx = np.nonzero(mask[b] != 0)[0]
            xs = np.zeros((C, D), dtype=np.float32)
            xs[: len(idx)] = qs[b][idx]
            mcol = np.zeros(C, dtype=np.float32)
            mcol[: len(idx)] = 1.0
        else:
            mcol = (mask[b] != 0).astype(np.float32)
            xs = qs[b] * mcol[:, None]
        xkv.append(np.ascontiguousarray(xs.T.astype(np_mm)))
        mc.append(mcol.astype(np_mm))

    in_maps = []
    for b in range(B):
        for g in range(HG):
            h0 = g * HL
            wq = Wqkv[:, (0 * H + h0) * A:(0 * H + h0 + HL) * A] * (
                1.0 / np.sqrt(np.float32(A))
            )
            wk = Wqkv[:, (1 * H + h0) * A:(1 * H + h0 + HL) * A]
            wv = Wqkv[:, (2 * H + h0) * A:(2 * H + h0 + HL) * A]
            wqkv_s = np.ascontiguousarray(
                np.concatenate([wq, wk, wv], axis=1,
                               dtype=np.float32).astype(np_mm)
            )
            wout_s = np.ascontiguousarray(
                Wout[h0 * A:(h0 + HL) * A, :].astype(np_mm)
            )
            in_maps.append(
                {
                    "xq": xq[b],
                    "xkv": xkv[b],
                    "wqkv": wqkv_s,
                    "wout": wout_s,
                    "mcol": mc[b],
                }
            )
    return in_maps, C


def gather_output(results, bout):
    """Sum the 4 head-group partials per batch (f32) and add bout."""
    out = np.empty((B, S, D), dtype=np.float32)
    for b in range(B):
        acc = results[b * HG]["out"].astype(np.float32)
        for g in range(1, HG):
            acc = acc + results[b * HG + g]["out"].astype(np.float32)
        out[b] = acc + bout.astype(np.float32)[None, :]
    return out


def _ensure_ntff_hook():
    """Inject antenv.axon_hooks (missing on this image) so trace=True works."""
    import sys
    import types

    try:
        from antenv import axon_hooks  # noqa: F401
        return
    except ImportError:
        pass
    mod = types.ModuleType("antenv.axon_hooks")
    _h = [None]
    mod.set_axon_ntff_profile_hook = lambda h: _h.__setitem__(0, h)
    mod.get_axon_ntff_profile_hook = lambda: _h[0]
    sys.modules["antenv.axon_hooks"] = mod
    import antenv

    antenv.axon_hooks = mod
    try:
        from trn_agent_boot.trn_boot import _ntff_profile_via_ctypes

        mod.set_axon_ntff_profile_hook(
            _ntff_profile_via_ctypes("/opt/axon/libaxon_pjrt.so")
        )
    except Exception:
        pass


def run(qs, mask, Wqkv, Wout, bout, trace=False):
    if trace:
        _ensure_ntff_hook()
    in_maps, C = prepare_in_maps(qs, mask, Wqkv, Wout)
    nc = _get_program(C)
    res = run_bass_kernel_spmd(
        nc, in_maps, core_ids=list(range(B * HG)), trace=trace
    )
    return gather_output(res.results, np.asarray(bout)), res


def kernel(qs, mask, Wqkv, Wout, bout):
    return run(qs, mask, Wqkv, Wout, bout, trace=False)[0]


# revision 13
# speedup vs baseline: 1.0779x; 1.0779x over previous
"""Multi-head attention (B=2, S=2048, D=1024, H=16, A=64) on 8 TRN2 NeuronCores.

Sharding: core c = b*4 + g handles batch b and head-group g (4 heads).
Tensor-parallel over heads; the all-reduce over head groups is host-side
during the gather (sum of 4 bf16 partials per batch, f32 accumulate).

Key structure (v2 — exp-stream-centric schedule):
 - The Scalar (ACT) engine's exp stream is the critical path (~9.4M exps/core).
   The schedule starts that stream as early as possible and keeps it gapless:
   scores for one head pair land in two alternating PSUM tiles (sc_e/sc_o)
   that exp drains back-to-back while the PE refills the other; everything
   else (projections, PV, out-proj) is PE filler emitted between score
   matmuls in dependency-safe program order.
 - Masking is folded into the data, not the exp: invalid/padded key columns
   of xkv are zeroed host-side (k=0 -> score 0 -> exp 1 -> v=0 contributes
   nothing) and the softmax denominator comes from a 0/1 mask column
   appended to V, so masked keys count 0 in the denominator too.
 - Scores for the two heads of a pair are emitted interleaved (rows 0:64 via
   PE tile T0, rows 64:128 via T8) so the row-tiled PE can overlap them.
 - ~140 dummy warm-up matmuls at t=0 keep the PE busy while input DMAs land,
   flipping the HAM clock gate to 2.4 GHz before real work starts.
 - Output projection streams per 128-query tile into bf16 DRAM as soon as
   each half's context is normalized; host gather sums partials in f32.
"""

import numpy as np

import concourse.bass as bass
import concourse.bacc as bacc
import concourse.mybir as mybir
import concourse.tile as tile
from concourse.bass_utils import run_bass_kernel_spmd

F32 = mybir.dt.float32
BF16 = mybir.dt.bfloat16
DT_MM = BF16
EXP = mybir.ActivationFunctionType.Exp

B = 2
S = 2048
D = 1024
H = 16
A = 64
HG = 4           # head groups (cores per batch)
HL = H // HG     # heads per core = 4
DSUB = D // 128  # 8
QCH = 512        # matmul qi chunk
HALF = 1024      # qi half processed per quarter


def build_program(C: int) -> bass.Bass:
    """Per-core SPMD program for key-capacity C (multiple of 128)."""
    KT = C // 128
    # kv windows: first is 128 keys so scores can start early
    kwins = [(0, 128)]
    pos = 128
    while pos < C:
        w = min(QCH, C - pos)
        kwins.append((pos, w))
        pos += w
    n_pr_bufs = 24 if KT > 10 else min(44, 4 * KT + 2)

    nc = bacc.Bacc("TRN2", target_bir_lowering=False, name=f"mha2_c{C}")
    xq_d = nc.dram_tensor("xq", [D, S], DT_MM, kind="ExternalInput")
    xkv_d = nc.dram_tensor("xkv", [D, C], DT_MM, kind="ExternalInput")
    wqkv_d = nc.dram_tensor("wqkv", [D, 3 * HL * A], DT_MM, kind="ExternalInput")
    wout_d = nc.dram_tensor("wout", [HL * A, D], DT_MM, kind="ExternalInput")
    mcol_d = nc.dram_tensor("mcol", [C], DT_MM, kind="ExternalInput")
    out_d = nc.dram_tensor("out", [S, D], BF16, kind="ExternalOutput")

    with tile.TileContext(nc) as tc:
        with (
            tc.tile_pool(name="const", bufs=1) as const,
            tc.tile_pool(name="probs", bufs=n_pr_bufs) as probs,
            tc.tile_pool(name="norm", bufs=3) as norm,
            tc.tile_pool(name="pvcp", bufs=5) as pvcp,
            tc.tile_pool(name="dramp", bufs=4, space="DRAM") as dramp,
            tc.tile_pool(name="outp", bufs=3) as outp,
            tc.tile_pool(name="psS", bufs=3, space="PSUM") as psS,
            tc.tile_pool(name="psW", bufs=2, space="PSUM") as psW,
        ):
            # ---- persistent SBUF residents ----
            w_sb = const.tile([128, DSUB, 3 * HL * A], DT_MM)
            wout_sb = const.tile([128, 2, D], DT_MM)
            qT = const.tile([128, 2, S], DT_MM)        # [hp*64+a, h2, qi]
            kT = const.tile([128, 2, C], DT_MM)        # [hp*64+a, h2, ki]
            vx = const.tile([128, KT, HL, A + 1], DT_MM)  # [ki%128, kt, h, a|mask]
            ctxT = const.tile([128, 2, S], DT_MM)      # [hp*64+a, h2, qi]
            warm = const.tile([128, 128], DT_MM)
            xq_t = const.tile([128, DSUB, S], DT_MM)
            xkva = const.tile([128, DSUB, 128], DT_MM)
            xkvb = (const.tile([128, DSUB, C - 128], DT_MM, name="xkvb")
                    if C > 128 else None)

            nc.vector.memset(warm, 0.0)

            w_r = wqkv_d.ap().rearrange("(o p) j -> p o j", p=128)
            xq_r = xq_d.ap().rearrange("(o p) s -> p o s", p=128)
            xkv_r = xkv_d.ap().rearrange("(o p) s -> p o s", p=128)
            mcol_r = mcol_d.ap().rearrange("(t p) -> p t", p=128)

            # ---- DMA issue order (earliest-needed first) ----
            nc.sync.dma_start(w_sb[:, :, 0:256], w_r[:, :, 0:256])      # wq
            nc.sync.dma_start(xq_t[:, :, 0:512], xq_r[:, :, 0:512])
            nc.sync.dma_start(w_sb[:, :, 256:512], w_r[:, :, 256:512])  # wk
            nc.sync.dma_start(xkva, xkv_r[:, :, 0:128])
            if len(kwins) > 1:
                b1, w1 = kwins[1]
                nc.sync.dma_start(xkvb[:, :, 0:w1], xkv_r[:, :, b1:b1 + w1])
            nc.sync.dma_start(xq_t[:, :, 512:1024], xq_r[:, :, 512:1024])
            for base, w in kwins[2:]:
                nc.sync.dma_start(xkvb[:, :, base - 128:base - 128 + w],
                                  xkv_r[:, :, base:base + w])
            nc.sync.dma_start(w_sb[:, :, 512:768], w_r[:, :, 512:768])  # wv
            for h in range(HL):
                nc.sync.dma_start(vx[:, :, h, A], mcol_r)
            nc.sync.dma_start(xq_t[:, :, 1024:1536], xq_r[:, :, 1024:1536])
            nc.sync.dma_start(xq_t[:, :, 1536:2048], xq_r[:, :, 1536:2048])
            nc.sync.dma_start(
                wout_sb, wout_d.ap().rearrange("(s p) d -> p s d", p=128)
            )

            # ---- HAM warm-up: keep PE busy while input DMAs land ----
            wp = psW.tile([128, 512], F32, tag="w", name="warmps")
            for _ in range(64):
                nc.tensor.matmul(wp[:, 0:128], warm, warm, start=True, stop=True)

            # ---- emission helpers (each is one PE "filler unit") ----
            def qproj_ct(c, ct):
                ps = psW.tile([128, 512], F32, tag="w", name=f"qp{c}_{ct}")
                for o in range(DSUB):
                    nc.tensor.matmul(
                        ps,
                        w_sb[:, o, ct * 128:(ct + 1) * 128],
                        xq_t[:, o, c * QCH:(c + 1) * QCH],
                        start=(o == 0),
                        stop=(o == DSUB - 1),
                    )
                nc.vector.tensor_copy(out=qT[:, ct, c * QCH:(c + 1) * QCH],
                                      in_=ps)

            def xkv_ap(base, w):
                if base == 0:
                    assert w <= 128
                    return xkva[:, :, 0:w]
                return xkvb[:, :, base - 128:base - 128 + w]

            def kproj_ct(wi, ct):
                base, w = kwins[wi]
                src = xkv_ap(base, w)
                ps = psW.tile([128, 512], F32, tag="w", name=f"kp{wi}_{ct}")
                for o in range(DSUB):
                    nc.tensor.matmul(
                        ps[:, :w],
                        w_sb[:, o, 256 + ct * 128:256 + (ct + 1) * 128],
                        src[:, o, :],
                        start=(o == 0),
                        stop=(o == DSUB - 1),
                    )
                nc.vector.tensor_copy(out=kT[:, ct, base:base + w],
                                      in_=ps[:, :w])

            def vproj_kt(kt):
                src = xkv_ap(kt * 128, 128)
                ps = psW.tile([128, 512], F32, tag="w", name=f"vp{kt}")
                pv = ps[:, 0:HL * A]
                for o in range(DSUB):
                    nc.tensor.matmul(
                        pv,
                        src[:, o, :],
                        w_sb[:, o, 512:512 + HL * A],
                        start=(o == 0),
                        stop=(o == DSUB - 1),
                    )
                nc.vector.tensor_copy(
                    out=vx[:, kt, :, 0:A],
                    in_=pv.rearrange("p (h a) -> p h a", a=A),
                )

            prs = {}

            def scores_pair(half, h2, kt):
                """Interleaved T0/T8 score matmuls + 2 exps for a head pair."""
                q0 = half * HALF
                sc_e = psS.tile([128, HALF], F32, tag="sc",
                                name=f"sce{half}_{h2}_{kt}")
                sc_o = psS.tile([128, HALF], F32, tag="sc",
                                name=f"sco{half}_{h2}_{kt}")
                for cc in range(2):
                    qs_ = slice(q0 + cc * QCH, q0 + (cc + 1) * QCH)
                    cs = slice(cc * QCH, (cc + 1) * QCH)
                    nc.tensor.matmul(
                        sc_e[:, cs],
                        kT[0:64, h2, kt * 128:(kt + 1) * 128],
                        qT[0:64, h2, qs_],
                        start=True, stop=True,
                    )
                    nc.tensor.matmul(
                        sc_o[:, cs],
                        kT[64:128, h2, kt * 128:(kt + 1) * 128],
                        qT[64:128, h2, qs_],
                        start=True, stop=True,
                    )
                pr_e = probs.tile([128, HALF], DT_MM, tag="pr",
                                  name=f"pre{half}_{h2}_{kt}")
                pr_o = probs.tile([128, HALF], DT_MM, tag="pr",
                                  name=f"pro{half}_{h2}_{kt}")
                nc.scalar.activation(out=pr_e, in_=sc_e, func=EXP)
                nc.scalar.activation(out=pr_o, in_=sc_o, func=EXP)
                prs[(half, 2 * h2, kt)] = pr_e
                prs[(half, 2 * h2 + 1, kt)] = pr_o

            def pv_unit(h, half, cl):
                """PV accumulation for (head, half, chunk-in-half cl).
                Early-drains PSUM to SBUF (pvc) so the bank frees fast.
                Returns a closure finishing the normalize chain."""
                hp, h2 = h % 2, h // 2
                pvt = psW.tile([128, 512], F32, tag="w",
                               name=f"pv{h}_{half}_{cl}")
                pva = pvt[0:A + 1, :]
                for kt in range(KT):
                    nc.tensor.matmul(
                        pva,
                        vx[:, kt, h, :],
                        prs[(half, h, kt)][:, cl * QCH:(cl + 1) * QCH],
                        start=(kt == 0),
                        stop=(kt == KT - 1),
                    )
                pvc = pvcp.tile([65, 512], F32, tag="pvc",
                                name=f"pvc{h}_{half}_{cl}")
                nc.vector.tensor_copy(out=pvc, in_=pva)
                dnd = dramp.tile([1, 512], F32, tag="dnd")
                nc.gpsimd.dma_start(dnd, pvc[64:65, :])
                rBr = norm.tile([64, 512], F32, tag="rBr")
                dnd_b = bass.AP(
                    tensor=dnd.tensor,
                    offset=dnd.offset,
                    ap=[[0, 64], list(dnd.ap[1])],
                )
                nc.gpsimd.dma_start(rBr, dnd_b)

                cslice = slice(half * HALF + cl * QCH,
                               half * HALF + (cl + 1) * QCH)

                def finish():
                    rB = norm.tile([64, 512], F32, tag="rB")
                    nc.vector.reciprocal_approx_fast(rB, rBr)
                    if hp == 0:
                        nc.vector.tensor_tensor(
                            ctxT[0:64, h2, cslice], pvc[0:A, :], rB,
                            mybir.AluOpType.mult,
                        )
                    else:
                        stg = norm.tile([64, 512], DT_MM, tag="stg")
                        nc.vector.tensor_tensor(
                            stg, pvc[0:A, :], rB, mybir.AluOpType.mult
                        )
                        nc.gpsimd.dma_start(ctxT[64:128, h2, cslice], stg)

                return finish

            def outproj_st(st, tail=False):
                """Output projection + bf16 drain + DMA for query tile st.
                In the tail the exp stream is done, so the Scalar engine
                drains one half to halve the DVE serialization."""
                ot = outp.tile([128, D], BF16, tag="ot", name=f"ot{st}")
                for dc in range(2):
                    po = psW.tile([128, 512], F32, tag="w", name=f"po{st}_{dc}")
                    for s2 in range(2):
                        nc.tensor.matmul(
                            po,
                            ctxT[:, s2, st * 128:(st + 1) * 128],
                            wout_sb[:, s2, dc * 512:(dc + 1) * 512],
                            start=(s2 == 0),
                            stop=(s2 == 1),
                        )
                    if tail and dc == 0:
                        nc.scalar.copy(
                            out=ot[:, dc * 512:(dc + 1) * 512], in_=po
                        )
                    else:
                        nc.vector.tensor_copy(
                            out=ot[:, dc * 512:(dc + 1) * 512], in_=po
                        )
                nc.sync.dma_start(out_d.ap()[st * 128:(st + 1) * 128, :], ot)

            # ---- lead-in PE work (minimum needed for first scores) ----
            qproj_ct(0, 0)
            qproj_ct(1, 0)
            kproj_ct(0, 0)

            pending_finish = []

            def run_unit(u):
                # flush one staged normalize-finish before each filler unit so
                # the DVE queue isn't head-of-line blocked on the broadcast DMA
                if pending_finish:
                    pending_finish.pop(0)()
                kind = u[0]
                if kind == "q":
                    qproj_ct(u[1], u[2])
                elif kind == "k":
                    kproj_ct(u[1], u[2])
                elif kind == "v":
                    vproj_kt(u[1])
                elif kind == "pv":
                    pending_finish.append(pv_unit(u[1], u[2], u[3]))
                elif kind == "st":
                    outproj_st(u[1])
                elif kind == "stt":
                    outproj_st(u[1], tail=True)

            # ---- per-quarter filler slots (emitted AFTER each slot's scores;
            #      every unit must precede, in PE order, the first score matmul
            #      that depends on it) ----
            def build_slots(units, KT):
                slots = [[] for _ in range(KT)]
                for i, u in enumerate(units):
                    slots[min(KT - 1, 1 + i * (KT - 1) // max(1, len(units)))
                          ].append(u)
                return slots

            # quarter (0,0): k-proj ct0 windows must be emitted before the
            # first e-score slot of their keys; ct1 before quarter (0,1);
            # q-ct1 and v-proj spread behind
            q00 = [[] for _ in range(KT)]
            pre_units = {}  # (half,h2,kt) -> units emitted just before its scores
            for wi in range(1, len(kwins)):
                pre_units[(0, 0, kwins[wi][0] // 128)] = [("k", wi, 0)]
                q00[min(3 + wi, KT - 1)].append(("k", wi, 1))
            q00[min(1, KT - 1)].append(("k", 0, 1))
            rest = [("q", 0, 1), ("q", 1, 1)] + [("v", kt) for kt in range(KT)]
            for i, u in enumerate(rest):
                q00[min(KT - 1, 2 + i * (KT - 3) // max(1, len(rest)))].append(u)

            quarter_slots = {
                (0, 0): q00,
                (0, 1): build_slots(
                    [("q", 2, 0), ("q", 2, 1), ("q", 3, 0), ("q", 3, 1),
                     ("pv", 0, 0, 0), ("pv", 1, 0, 0),
                     ("pv", 0, 0, 1), ("pv", 1, 0, 1)], KT),
                (1, 0): build_slots(
                    [("pv", 2, 0, 0), ("pv", 3, 0, 0), ("st", 0),
                     ("pv", 2, 0, 1), ("pv", 3, 0, 1),
                     ("st", 1), ("st", 2), ("st", 3),
                     ("st", 4), ("st", 5), ("st", 6), ("st", 7)], KT),
                (1, 1): build_slots(
                    [("pv", 0, 1, 0), ("pv", 1, 1, 0),
                     ("pv", 0, 1, 1), ("pv", 1, 1, 1)], KT),
            }

            # scores are emitted one slot AHEAD of fillers so the PE always
            # has the next pair of score matmuls queued while exp streams
            slot_seq = [(half, h2, kt)
                        for half in range(2) for h2 in range(2)
                        for kt in range(KT)]
            scores_pair(*slot_seq[0])
            for i, (half, h2, kt) in enumerate(slot_seq):
                if i + 1 < len(slot_seq):
                    nxt = slot_seq[i + 1]
                    for u in pre_units.get(nxt, []):
                        run_unit(u)
                    scores_pair(*nxt)
                for u in quarter_slots[(half, h2)][kt]:
                    run_unit(u)

            # ---- tail: last pair's PV + second-half out-proj ----
            for u in [("pv", 2, 1, 0), ("pv", 3, 1, 0),
                      ("stt", 8), ("stt", 9), ("stt", 10), ("stt", 11),
                      ("pv", 2, 1, 1), ("pv", 3, 1, 1),
                      ("stt", 12), ("stt", 13), ("stt", 14), ("stt", 15)]:
                run_unit(u)
            while pending_finish:
                pending_finish.pop(0)()

    return nc


_PROGRAM_CACHE: dict[int, bass.Bass] = {}


def _get_program(C: int) -> bass.Bass:
    if C not in _PROGRAM_CACHE:
        nc = build_program(C)
        nc.finalize()
        _PROGRAM_CACHE[C] = nc
    return _PROGRAM_CACHE[C]


def _ceil128(n: int) -> int:
    return max(128, (n + 127) // 128 * 128)


def prepare_in_maps(qs, mask, Wqkv, Wout):
    """Shard FULL inputs into 8 per-core input maps. Returns (in_maps, C)."""
    import ml_dtypes

    np_mm = ml_dtypes.bfloat16
    qs = np.ascontiguousarray(qs, dtype=np.float32)
    mask = np.asarray(mask)
    Wqkv = np.ascontiguousarray(Wqkv, dtype=np.float32)
    Wout = np.ascontiguousarray(Wout, dtype=np.float32)

    nvalid = [int(np.count_nonzero(mask[b])) for b in range(B)]
    if min(nvalid) == 0:
        C = S  # degenerate masks: run dense
    else:
        C = min(S, _ceil128(max(nvalid)))
    compact = C < S

    xq, xkv, mc = [], [], []
    for b in range(B):
        xq.append(np.ascontiguousarray(qs[b].T.astype(np_mm)))
        if compact:
            idx = np.nonzero(mask[b] != 0)[0]
            xs = np.zeros((C, D), dtype=np.float32)
            xs[: len(idx)] = qs[b][idx]
            mcol = np.zeros(C, dtype=np.float32)
            mcol[: len(idx)] = 1.0
        else:
            mcol = (mask[b] != 0).astype(np.float32)
            xs = qs[b] * mcol[:, None]
        xkv.append(np.ascontiguousarray(xs.T.astype(np_mm)))
        mc.append(mcol.astype(np_mm))

    in_maps = []
    for b in range(B):
        for g in range(HG):
            h0 = g * HL
            wq = Wqkv[:, (0 * H + h0) * A:(0 * H + h0 + HL) * A] * (
                1.0 / np.sqrt(np.float32(A))
            )
            wk = Wqkv[:, (1 * H + h0) * A:(1 * H + h0 + HL) * A]
            wv = Wqkv[:, (2 * H + h0) * A:(2 * H + h0 + HL) * A]
            wqkv_s = np.ascontiguousarray(
                np.concatenate([wq, wk, wv], axis=1,
                               dtype=np.float32).astype(np_mm)
            )
            wout_s = np.ascontiguousarray(
                Wout[h0 * A:(h0 + HL) * A, :].astype(np_mm)
            )
            in_maps.append(
                {
                    "xq": xq[b],
                    "xkv": xkv[b],
                    "wqkv": wqkv_s,
                    "wout": wout_s,
                    "mcol": mc[b],
                }
            )
    return in_maps, C


def gather_output(results, bout):
    """Sum the 4 head-group partials per batch (f32) and add bout."""
    out = np.empty((B, S, D), dtype=np.float32)
    for b in range(B):
        acc = results[b * HG]["out"].astype(np.float32)
        for g in range(1, HG):
            acc = acc + results[b * HG + g]["out"].astype(np.float32)
        out[b] = acc + bout.astype(np.float32)[None, :]
    return out


def _ensure_ntff_hook():
    """Inject antenv.axon_hooks (missing on this image) so trace=True works."""
    import sys
    import types

    try:
        from antenv import axon_hooks  # noqa: F401
        return
    except ImportError:
        pass
    mod = types.ModuleType("antenv.axon_hooks")
    _h = [None]
    mod.set_axon_ntff_profile_hook = lambda h: _h.__setitem__(0, h)
    mod.get_axon_ntff_profile_hook = lambda: _h[0]
    sys.modules["antenv.axon_hooks"] = mod
    import antenv

    antenv.axon_hooks = mod
    try:
        from trn_agent_boot.trn_boot import _ntff_profile_via_ctypes

        mod.set_axon_ntff_profile_hook(
            _ntff_profile_via_ctypes("/opt/axon/libaxon_pjrt.so")
        )
    except Exception:
        pass


def run(qs, mask, Wqkv, Wout, bout, trace=False):
    if trace:
        _ensure_ntff_hook()
    in_maps, C = prepare_in_maps(qs, mask, Wqkv, Wout)
    nc = _get_program(C)
    res = run_bass_kernel_spmd(
        nc, in_maps, core_ids=list(range(B * HG)), trace=trace
    )
    return gather_output(res.results, np.asarray(bout)), res


def kernel(qs, mask, Wqkv, Wout, bout):
    return run(qs, mask, Wqkv, Wout, bout, trace=False)[0]


# revision 16
# speedup vs baseline: 1.1260x; 1.0446x over previous
"""Multi-head attention (B=2, S=2048, D=1024, H=16, A=64) on 8 TRN2 NeuronCores.

Sharding: core c = b*4 + g handles batch b and head-group g (4 heads).
Tensor-parallel over heads; the all-reduce over head groups is host-side
during the gather (sum of 4 bf16 partials per batch, f32 accumulate).

Key structure (v2 — exp-stream-centric schedule):
 - The Scalar (ACT) engine's exp stream is the critical path (~9.4M exps/core).
   The schedule starts that stream as early as possible and keeps it gapless:
   scores for one head pair land in two alternating PSUM tiles (sc_e/sc_o)
   that exp drains back-to-back while the PE refills the other; everything
   else (projections, PV, out-proj) is PE filler emitted between score
   matmuls in dependency-safe program order.
 - Masking is folded into the data, not the exp: invalid/padded key columns
   of xkv are zeroed host-side (k=0 -> score 0 -> exp 1 -> v=0 contributes
   nothing) and the softmax denominator comes from a 0/1 mask column
   appended to V, so masked keys count 0 in the denominator too.
 - Scores for the two heads of a pair are emitted interleaved (rows 0:64 via
   PE tile T0, rows 64:128 via T8) so the row-tiled PE can overlap them.
 - ~140 dummy warm-up matmuls at t=0 keep the PE busy while input DMAs land,
   flipping the HAM clock gate to 2.4 GHz before real work starts.
 - Output projection streams per 128-query tile into bf16 DRAM as soon as
   each half's context is normalized; host gather sums partials in f32.
"""

import numpy as np

import concourse.bass as bass
import concourse.bacc as bacc
import concourse.mybir as mybir
import concourse.tile as tile
from concourse.bass_utils import run_bass_kernel_spmd

F32 = mybir.dt.float32
BF16 = mybir.dt.bfloat16
DT_MM = BF16
EXP = mybir.ActivationFunctionType.Exp

B = 2
S = 2048
D = 1024
H = 16
A = 64
HG = 4           # head groups (cores per batch)
HL = H // HG     # heads per core = 4
DSUB = D // 128  # 8
QCH = 512        # matmul qi chunk
HALF = 1024      # qi half processed per quarter


def build_program(C: int) -> bass.Bass:
    """Per-core SPMD program for key-capacity C (multiple of 128)."""
    KT = C // 128
    # kv windows: first is 128 keys so scores can start early
    kwins = [(0, 128)]
    pos = 128
    while pos < C:
        w = min(QCH, C - pos)
        kwins.append((pos, w))
        pos += w
    n_pr_bufs = 24 if KT > 10 else min(44, 4 * KT + 4)

    nc = bacc.Bacc("TRN2", target_bir_lowering=False, name=f"mha2_c{C}")
    xq_d = nc.dram_tensor("xq", [D, S], DT_MM, kind="ExternalInput")
    xkv_d = nc.dram_tensor("xkv", [D, C], DT_MM, kind="ExternalInput")
    wqkv_d = nc.dram_tensor("wqkv", [128, DSUB, 3 * HL * A], DT_MM,
                            kind="ExternalInput")
    wout_d = nc.dram_tensor("wout", [128, 2, D], DT_MM, kind="ExternalInput")
    mcol_d = nc.dram_tensor("mcol", [C], DT_MM, kind="ExternalInput")
    out_d = nc.dram_tensor("out", [S, D], BF16, kind="ExternalOutput")

    with tile.TileContext(nc) as tc:
        with (
            tc.tile_pool(name="const", bufs=1) as const,
            tc.tile_pool(name="probs", bufs=n_pr_bufs) as probs,
            tc.tile_pool(name="norm", bufs=3) as norm,
            tc.tile_pool(name="pvcp", bufs=5) as pvcp,
            tc.tile_pool(name="dramp", bufs=4, space="DRAM") as dramp,
            tc.tile_pool(name="outp", bufs=3) as outp,
            tc.tile_pool(name="psS", bufs=3, space="PSUM") as psS,
            tc.tile_pool(name="psW", bufs=2, space="PSUM") as psW,
        ):
            # ---- persistent SBUF residents ----
            w_sb = const.tile([128, DSUB, 3 * HL * A], DT_MM)
            wout_sb = const.tile([128, 2, D], DT_MM)
            qT = const.tile([128, 2, S], DT_MM)        # [hp*64+a, h2, qi]
            kT = const.tile([128, 2, C], DT_MM)        # [hp*64+a, h2, ki]
            vx = const.tile([128, KT, HL, A + 1], DT_MM)  # [ki%128, kt, h, a|mask]
            ctxT = const.tile([128, 2, S], DT_MM)      # [hp*64+a, h2, qi]
            warm = const.tile([128, 128], DT_MM)
            xq_t = const.tile([128, DSUB, S], DT_MM)
            xkva = const.tile([128, DSUB, 128], DT_MM)
            xkvb = (const.tile([128, DSUB, C - 128], DT_MM, name="xkvb")
                    if C > 128 else None)

            nc.vector.memset(warm, 0.0)

            xq_r = xq_d.ap().rearrange("(o p) s -> p o s", p=128)
            xkv_r = xkv_d.ap().rearrange("(o p) s -> p o s", p=128)
            mcol_r = mcol_d.ap().rearrange("(t p) -> p t", p=128)

            # ---- DMA issue order (earliest-needed first); x transfers are
            #      per-(o, s-range) so each DRAM descriptor is a 2KB+ run ----
            nc.sync.dma_start(w_sb, wqkv_d.ap())
            for o in range(DSUB):
                nc.sync.dma_start(xq_t[:, o, 0:1024], xq_r[:, o, 0:1024])
            nc.sync.dma_start(xkva, xkv_r[:, :, 0:128])
            for o in range(DSUB):
                nc.sync.dma_start(xkvb[:, o, :], xkv_r[:, o, 128:C])
            for o in range(DSUB):
                nc.sync.dma_start(xq_t[:, o, 1024:2048], xq_r[:, o, 1024:2048])
            for h in range(HL):
                nc.sync.dma_start(vx[:, :, h, A], mcol_r)
            nc.sync.dma_start(wout_sb, wout_d.ap())

            # ---- HAM warm-up: keep PE busy while input DMAs land ----
            wp = psW.tile([128, 512], F32, tag="w", name="warmps")
            for _ in range(64):
                nc.tensor.matmul(wp[:, 0:128], warm, warm, start=True, stop=True)

            # ---- emission helpers (each is one PE "filler unit") ----
            def qproj_ct(c, ct):
                ps = psW.tile([128, 512], F32, tag="w", name=f"qp{c}_{ct}")
                for o in range(DSUB):
                    nc.tensor.matmul(
                        ps,
                        w_sb[:, o, ct * 128:(ct + 1) * 128],
                        xq_t[:, o, c * QCH:(c + 1) * QCH],
                        start=(o == 0),
                        stop=(o == DSUB - 1),
                    )
                nc.vector.tensor_copy(out=qT[:, ct, c * QCH:(c + 1) * QCH],
                                      in_=ps)

            def xkv_ap(base, w):
                if base == 0:
                    assert w <= 128
                    return xkva[:, :, 0:w]
                return xkvb[:, :, base - 128:base - 128 + w]

            def kproj_ct(wi, ct):
                base, w = kwins[wi]
                src = xkv_ap(base, w)
                ps = psW.tile([128, 512], F32, tag="w", name=f"kp{wi}_{ct}")
                for o in range(DSUB):
                    nc.tensor.matmul(
                        ps[:, :w],
                        w_sb[:, o, 256 + ct * 128:256 + (ct + 1) * 128],
                        src[:, o, :],
                        start=(o == 0),
                        stop=(o == DSUB - 1),
                    )
                nc.vector.tensor_copy(out=kT[:, ct, base:base + w],
                                      in_=ps[:, :w])

            def vproj_kt(kt):
                src = xkv_ap(kt * 128, 128)
                ps = psW.tile([128, 512], F32, tag="w", name=f"vp{kt}")
                pv = ps[:, 0:HL * A]
                for o in range(DSUB):
                    nc.tensor.matmul(
                        pv,
                        src[:, o, :],
                        w_sb[:, o, 512:512 + HL * A],
                        start=(o == 0),
                        stop=(o == DSUB - 1),
                    )
                nc.vector.tensor_copy(
                    out=vx[:, kt, :, 0:A],
                    in_=pv.rearrange("p (h a) -> p h a", a=A),
                )

            prs = {}

            def scores_pair(half, h2, kt):
                """Interleaved T0/T8 score matmuls + 2 exps for a head pair."""
                q0 = half * HALF
                sc_e = psS.tile([128, HALF], F32, tag="sc",
                                name=f"sce{half}_{h2}_{kt}")
                sc_o = psS.tile([128, HALF], F32, tag="sc",
                                name=f"sco{half}_{h2}_{kt}")
                for cc in range(2):
                    qs_ = slice(q0 + cc * QCH, q0 + (cc + 1) * QCH)
                    cs = slice(cc * QCH, (cc + 1) * QCH)
                    nc.tensor.matmul(
                        sc_e[:, cs],
                        kT[0:64, h2, kt * 128:(kt + 1) * 128],
                        qT[0:64, h2, qs_],
                        start=True, stop=True,
                    )
                    nc.tensor.matmul(
                        sc_o[:, cs],
                        kT[64:128, h2, kt * 128:(kt + 1) * 128],
                        qT[64:128, h2, qs_],
                        start=True, stop=True,
                    )
                pr_e = probs.tile([128, HALF], DT_MM, tag="pr",
                                  name=f"pre{half}_{h2}_{kt}")
                pr_o = probs.tile([128, HALF], DT_MM, tag="pr",
                                  name=f"pro{half}_{h2}_{kt}")
                nc.scalar.activation(out=pr_e, in_=sc_e, func=EXP)
                nc.scalar.activation(out=pr_o, in_=sc_o, func=EXP)
                prs[(half, 2 * h2, kt)] = pr_e
                prs[(half, 2 * h2 + 1, kt)] = pr_o

            def pv_unit(h, half, cl, tail=False):
                """PV accumulation for (head, half, chunk-in-half cl).
                Early-drains PSUM to SBUF (pvc) so the bank frees fast.
                Returns a closure finishing the normalize chain."""
                hp, h2 = h % 2, h // 2
                pvt = psW.tile([128, 512], F32, tag="w",
                               name=f"pv{h}_{half}_{cl}")
                pva = pvt[0:A + 1, :]
                for kt in range(KT):
                    nc.tensor.matmul(
                        pva,
                        vx[:, kt, h, :],
                        prs[(half, h, kt)][:, cl * QCH:(cl + 1) * QCH],
                        start=(kt == 0),
                        stop=(kt == KT - 1),
                    )
                pvc = pvcp.tile([65, 512], F32, tag="pvc",
                                name=f"pvc{h}_{half}_{cl}")
                if tail:
                    nc.scalar.copy(out=pvc, in_=pva)
                else:
                    nc.vector.tensor_copy(out=pvc, in_=pva)
                dnd = dramp.tile([1, 512], F32, tag="dnd")
                nc.gpsimd.dma_start(dnd, pvc[64:65, :])
                rBr = norm.tile([64, 512], F32, tag="rBr")
                dnd_b = bass.AP(
                    tensor=dnd.tensor,
                    offset=dnd.offset,
                    ap=[[0, 64], list(dnd.ap[1])],
                )
                nc.gpsimd.dma_start(rBr, dnd_b)

                cslice = slice(half * HALF + cl * QCH,
                               half * HALF + (cl + 1) * QCH)

                def finish():
                    rB = norm.tile([64, 512], F32, tag="rB")
                    nc.vector.reciprocal_approx_fast(rB, rBr)
                    if hp == 0:
                        nc.vector.tensor_tensor(
                            ctxT[0:64, h2, cslice], pvc[0:A, :], rB,
                            mybir.AluOpType.mult,
                        )
                    else:
                        stg = norm.tile([64, 512], DT_MM, tag="stg")
                        nc.vector.tensor_tensor(
                            stg, pvc[0:A, :], rB, mybir.AluOpType.mult
                        )
                        nc.gpsimd.dma_start(ctxT[64:128, h2, cslice], stg)

                return finish

            def outproj_st(st, tail=False):
                """Output projection + bf16 drain + DMA for query tile st.
                In the tail the exp stream is done, so the Scalar engine
                drains one half to halve the DVE serialization."""
                ot = outp.tile([128, D], BF16, tag="ot", name=f"ot{st}")
                for dc in range(2):
                    po = psW.tile([128, 512], F32, tag="w", name=f"po{st}_{dc}")
                    for s2 in range(2):
                        nc.tensor.matmul(
                            po,
                            ctxT[:, s2, st * 128:(st + 1) * 128],
                            wout_sb[:, s2, dc * 512:(dc + 1) * 512],
                            start=(s2 == 0),
                            stop=(s2 == 1),
                        )
                    if tail and dc == 0:
                        nc.scalar.copy(
                            out=ot[:, dc * 512:(dc + 1) * 512], in_=po
                        )
                    else:
                        nc.vector.tensor_copy(
                            out=ot[:, dc * 512:(dc + 1) * 512], in_=po
                        )
                nc.sync.dma_start(out_d.ap()[st * 128:(st + 1) * 128, :], ot)

            # ---- lead-in PE work (minimum needed for first scores) ----
            qproj_ct(0, 0)
            qproj_ct(1, 0)
            kproj_ct(0, 0)

            pending_finish = []

            def run_unit(u):
                # flush one staged normalize-finish before each filler unit so
                # the DVE queue isn't head-of-line blocked on the broadcast DMA
                if pending_finish:
                    pending_finish.pop(0)()
                kind = u[0]
                if kind == "q":
                    qproj_ct(u[1], u[2])
                elif kind == "k":
                    kproj_ct(u[1], u[2])
                elif kind == "v":
                    vproj_kt(u[1])
                elif kind == "pv":
                    pending_finish.append(pv_unit(u[1], u[2], u[3]))
                elif kind == "pvt":
                    pending_finish.append(pv_unit(u[1], u[2], u[3], tail=True))
                elif kind == "st":
                    outproj_st(u[1])
                elif kind == "stt":
                    outproj_st(u[1], tail=True)

            # ---- per-quarter filler slots (emitted AFTER each slot's scores;
            #      every unit must precede, in PE order, the first score matmul
            #      that depends on it) ----
            def build_slots(units, KT):
                slots = [[] for _ in range(KT)]
                for i, u in enumerate(units):
                    slots[min(KT - 1, 1 + i * (KT - 1) // max(1, len(units)))
                          ].append(u)
                return slots

            # quarter (0,0): k-proj ct0 windows must be emitted before the
            # first e-score slot of their keys; ct1 before quarter (0,1);
            # q-ct1 and v-proj spread behind
            q00 = [[] for _ in range(KT)]
            pre_units = {}  # (half,h2,kt) -> units emitted just before its scores
            for wi in range(1, len(kwins)):
                pre_units[(0, 0, kwins[wi][0] // 128)] = [("k", wi, 0)]
                q00[min(3 + wi, KT - 1)].append(("k", wi, 1))
            q00[min(1, KT - 1)].append(("k", 0, 1))
            nv0 = max(0, KT - 2)  # v-proj kts kept in quarter (0,0)
            rest = [("q", 0, 1), ("q", 1, 1)] + [("v", kt) for kt in range(nv0)]
            for i, u in enumerate(rest):
                q00[min(KT - 1, 2 + i * (KT - 3) // max(1, len(rest)))].append(u)

            quarter_slots = {
                (0, 0): q00,
                (0, 1): build_slots(
                    [("v", kt) for kt in range(nv0, KT)] +
                    [("q", 2, 0), ("q", 3, 0),
                     ("pv", 0, 0, 0), ("pv", 1, 0, 0)], KT),
                (1, 0): build_slots(
                    [("pv", 0, 0, 1), ("pv", 1, 0, 1),
                     ("q", 2, 1), ("q", 3, 1),
                     ("pv", 2, 0, 0), ("pv", 3, 0, 0), ("st", 0),
                     ("pv", 2, 0, 1), ("pv", 3, 0, 1), ("st", 1)], KT),
                (1, 1): build_slots(
                    [("st", 2), ("st", 3), ("st", 4), ("st", 5),
                     ("st", 6), ("st", 7),
                     ("pv", 0, 1, 0), ("pv", 1, 1, 0),
                     ("pv", 0, 1, 1), ("pv", 1, 1, 1)], KT),
            }

            # scores are emitted one slot AHEAD of fillers so the PE always
            # has the next pair of score matmuls queued while exp streams
            slot_seq = [(half, h2, kt)
                        for half in range(2) for h2 in range(2)
                        for kt in range(KT)]
            scores_pair(*slot_seq[0])
            for i, (half, h2, kt) in enumerate(slot_seq):
                if i + 1 < len(slot_seq):
                    nxt = slot_seq[i + 1]
                    for u in pre_units.get(nxt, []):
                        run_unit(u)
                    scores_pair(*nxt)
                for u in quarter_slots[(half, h2)][kt]:
                    run_unit(u)

            # ---- tail: last pair's PV + second-half out-proj ----
            for u in [("pvt", 2, 1, 0), ("pvt", 3, 1, 0),
                      ("stt", 8), ("stt", 9), ("stt", 10), ("stt", 11),
                      ("pvt", 2, 1, 1), ("pvt", 3, 1, 1),
                      ("stt", 12), ("stt", 13), ("stt", 14), ("stt", 15)]:
                run_unit(u)
            while pending_finish:
                pending_finish.pop(0)()

    return nc


_PROGRAM_CACHE: dict[int, bass.Bass] = {}


def _get_program(C: int) -> bass.Bass:
    if C not in _PROGRAM_CACHE:
        nc = build_program(C)
        nc.finalize()
        _PROGRAM_CACHE[C] = nc
    return _PROGRAM_CACHE[C]


def _ceil128(n: int) -> int:
    return max(128, (n + 127) // 128 * 128)


def prepare_in_maps(qs, mask, Wqkv, Wout):
    """Shard FULL inputs into 8 per-core input maps. Returns (in_maps, C)."""
    import ml_dtypes

    np_mm = ml_dtypes.bfloat16
    qs = np.ascontiguousarray(qs, dtype=np.float32)
    mask = np.asarray(mask)
    Wqkv = np.ascontiguousarray(Wqkv, dtype=np.float32)
    Wout = np.ascontiguousarray(Wout, dtype=np.float32)

    nvalid = [int(np.count_nonzero(mask[b])) for b in range(B)]
    if min(nvalid) == 0:
        C = S  # degenerate masks: run dense
    else:
        C = min(S, _ceil128(max(nvalid)))
    compact = C < S

    xq, xkv, mc = [], [], []
    for b in range(B):
        xq.append(np.ascontiguousarray(qs[b].T.astype(np_mm)))
        if compact:
            idx = np.nonzero(mask[b] != 0)[0]
            xs = np.zeros((C, D), dtype=np.float32)
            xs[: len(idx)] = qs[b][idx]
            mcol = np.zeros(C, dtype=np.float32)
            mcol[: len(idx)] = 1.0
        else:
            mcol = (mask[b] != 0).astype(np.float32)
            xs = qs[b] * mcol[:, None]
        xkv.append(np.ascontiguousarray(xs.T.astype(np_mm)))
        mc.append(mcol.astype(np_mm))

    in_maps = []
    for b in range(B):
        for g in range(HG):
            h0 = g * HL
            wq = Wqkv[:, (0 * H + h0) * A:(0 * H + h0 + HL) * A] * (
                1.0 / np.sqrt(np.float32(A))
            )
            wk = Wqkv[:, (1 * H + h0) * A:(1 * H + h0 + HL) * A]
            wv = Wqkv[:, (2 * H + h0) * A:(2 * H + h0 + HL) * A]
            wcat = np.concatenate([wq, wk, wv], axis=1, dtype=np.float32)
            # device layout [128, DSUB, 768]: row o*128+p -> [p, o, :]
            wqkv_s = np.ascontiguousarray(
                wcat.reshape(DSUB, 128, 3 * HL * A).transpose(1, 0, 2)
                .astype(np_mm)
            )
            # device layout [128, 2, D]: row s2*128+p -> [p, s2, :]
            wout_s = np.ascontiguousarray(
                Wout[h0 * A:(h0 + HL) * A, :]
                .reshape(2, 128, D).transpose(1, 0, 2).astype(np_mm)
            )
            in_maps.append(
                {
                    "xq": xq[b],
                    "xkv": xkv[b],
                    "wqkv": wqkv_s,
                    "wout": wout_s,
                    "mcol": mc[b],
                }
            )
    return in_maps, C


def gather_output(results, bout):
    """Sum the 4 head-group partials per batch (f32) and add bout."""
    out = np.empty((B, S, D), dtype=np.float32)
    for b in range(B):
        acc = results[b * HG]["out"].astype(np.float32)
        for g in range(1, HG):
            acc = acc + results[b * HG + g]["out"].astype(np.float32)
        out[b] = acc + bout.astype(np.float32)[None, :]
    return out


def _ensure_ntff_hook():
    """Inject antenv.axon_hooks (missing on this image) so trace=True works."""
    import sys
    import types

    try:
        from antenv import axon_hooks  # noqa: F401
        return
    except ImportError:
        pass
    mod = types.ModuleType("antenv.axon_hooks")
    _h = [None]
    mod.set_axon_ntff_profile_hook = lambda h: _h.__setitem__(0, h)
    mod.get_axon_ntff_profile_hook = lambda: _h[0]
    sys.modules["antenv.axon_hooks"] = mod
    import antenv

    antenv.axon_hooks = mod
    try:
        from trn_agent_boot.trn_boot import _ntff_profile_via_ctypes

        mod.set_axon_ntff_profile_hook(
            _ntff_profile_via_ctypes("/opt/axon/libaxon_pjrt.so")
        )
    except Exception:
        pass


def run(qs, mask, Wqkv, Wout, bout, trace=False):
    if trace:
        _ensure_ntff_hook()
    in_maps, C = prepare_in_maps(qs, mask, Wqkv, Wout)
    nc = _get_program(C)
    res = run_bass_kernel_spmd(
        nc, in_maps, core_ids=list(range(B * HG)), trace=trace
    )
    return gather_output(res.results, np.asarray(bout)), res


def kernel(qs, mask, Wqkv, Wout, bout):
    return run(qs, mask, Wqkv, Wout, bout, trace=False)[0]


# revision 17
# speedup vs baseline: 1.2029x; 1.0683x over previous
"""Multi-head attention (B=2, S=2048, D=1024, H=16, A=64) on 8 TRN2 NeuronCores.

Sharding: core c = b*4 + g handles batch b and head-group g (4 heads).
Tensor-parallel over heads; the all-reduce over head groups is host-side
during the gather (sum of 4 bf16 partials per batch, f32 accumulate).

Key structure (v2 — exp-stream-centric schedule):
 - The Scalar (ACT) engine's exp stream is the critical path (~9.4M exps/core).
   The schedule starts that stream as early as possible and keeps it gapless:
   scores for one head pair land in two alternating PSUM tiles (sc_e/sc_o)
   that exp drains back-to-back while the PE refills the other; everything
   else (projections, PV, out-proj) is PE filler emitted between score
   matmuls in dependency-safe program order.
 - Masking is folded into the data, not the exp: invalid/padded key columns
   of xkv are zeroed host-side (k=0 -> score 0 -> exp 1 -> v=0 contributes
   nothing) and the softmax denominator comes from a 0/1 mask column
   appended to V, so masked keys count 0 in the denominator too.
 - Scores for the two heads of a pair are emitted interleaved (rows 0:64 via
   PE tile T0, rows 64:128 via T8) so the row-tiled PE can overlap them.
 - ~140 dummy warm-up matmuls at t=0 keep the PE busy while input DMAs land,
   flipping the HAM clock gate to 2.4 GHz before real work starts.
 - Output projection streams per 128-query tile into bf16 DRAM as soon as
   each half's context is normalized; host gather sums partials in f32.
"""

import numpy as np

import concourse.bass as bass
import concourse.bacc as bacc
import concourse.mybir as mybir
import concourse.tile as tile
from concourse.bass_utils import run_bass_kernel_spmd

F32 = mybir.dt.float32
BF16 = mybir.dt.bfloat16
DT_MM = BF16
EXP = mybir.ActivationFunctionType.Exp

B = 2
S = 2048
D = 1024
H = 16
A = 64
HG = 4           # head groups (cores per batch)
HL = H // HG     # heads per core = 4
DSUB = D // 128  # 8
QCH = 512        # matmul qi chunk
HALF = 1024      # qi half processed per quarter


def build_program(C: int) -> bass.Bass:
    """Per-core SPMD program for key-capacity C (multiple of 128)."""
    KT = C // 128
    # kv windows: first is 128 keys so scores can start early
    kwins = [(0, 128)]
    pos = 128
    while pos < C:
        w = min(QCH, C - pos)
        kwins.append((pos, w))
        pos += w
    n_pr_bufs = 24 if KT > 10 else min(44, 4 * KT + 4)

    nc = bacc.Bacc("TRN2", target_bir_lowering=False, name=f"mha2_c{C}")
    xq_d = nc.dram_tensor("xq", [D, S], DT_MM, kind="ExternalInput")
    xkv_d = nc.dram_tensor("xkv", [D, C], DT_MM, kind="ExternalInput")
    wqkv_d = nc.dram_tensor("wqkv", [128, DSUB, 3 * HL * A], DT_MM,
                            kind="ExternalInput")
    wout_d = nc.dram_tensor("wout", [128, 2, D], DT_MM, kind="ExternalInput")
    mcol_d = nc.dram_tensor("mcol", [C], DT_MM, kind="ExternalInput")
    out_d = nc.dram_tensor("out", [S, D], BF16, kind="ExternalOutput")

    with tile.TileContext(nc) as tc:
        with (
            tc.tile_pool(name="const", bufs=1) as const,
            tc.tile_pool(name="probs", bufs=n_pr_bufs) as probs,
            tc.tile_pool(name="norm", bufs=3) as norm,
            tc.tile_pool(name="pvcp", bufs=5) as pvcp,
            tc.tile_pool(name="dramp", bufs=4, space="DRAM") as dramp,
            tc.tile_pool(name="outp", bufs=3) as outp,
            tc.tile_pool(name="psS", bufs=3, space="PSUM") as psS,
            tc.tile_pool(name="psW", bufs=2, space="PSUM") as psW,
        ):
            # ---- persistent SBUF residents ----
            w_sb = const.tile([128, DSUB, 3 * HL * A], DT_MM)
            wout_sb = const.tile([128, 2, D], DT_MM)
            qT = const.tile([128, 2, S], DT_MM)        # [hp*64+a, h2, qi]
            kT = const.tile([128, 2, C], DT_MM)        # [hp*64+a, h2, ki]
            vx = const.tile([128, KT, HL, A + 1], DT_MM)  # [ki%128, kt, h, a|mask]
            ctxT = const.tile([128, 2, S], DT_MM)      # [hp*64+a, h2, qi]
            warm = const.tile([128, 128], DT_MM)
            xq_t = const.tile([128, DSUB, S], DT_MM)
            xkva = const.tile([128, DSUB, 128], DT_MM)
            xkvb = (const.tile([128, DSUB, C - 128], DT_MM, name="xkvb")
                    if C > 128 else None)

            nc.vector.memset(warm, 0.0)

            xq_r = xq_d.ap().rearrange("(o p) s -> p o s", p=128)
            xkv_r = xkv_d.ap().rearrange("(o p) s -> p o s", p=128)
            mcol_r = mcol_d.ap().rearrange("(t p) -> p t", p=128)

            # ---- DMA issue order (earliest-needed first); x transfers are
            #      per-(o, s-range) so each DRAM descriptor is a 2KB+ run ----
            nc.sync.dma_start(w_sb, wqkv_d.ap())
            for o in range(DSUB):
                nc.sync.dma_start(xq_t[:, o, 0:1024], xq_r[:, o, 0:1024])
            nc.sync.dma_start(xkva, xkv_r[:, :, 0:128])
            xsplit = min(640, C)
            for o in range(DSUB):
                nc.sync.dma_start(xkvb[:, o, 0:xsplit - 128],
                                  xkv_r[:, o, 128:xsplit])
            if C > 640:
                for o in range(DSUB):
                    nc.sync.dma_start(xkvb[:, o, 512:C - 128],
                                      xkv_r[:, o, 640:C])
            for o in range(DSUB):
                nc.sync.dma_start(xq_t[:, o, 1024:2048], xq_r[:, o, 1024:2048])
            for h in range(HL):
                nc.sync.dma_start(vx[:, :, h, A], mcol_r)
            nc.sync.dma_start(wout_sb, wout_d.ap())

            # ---- HAM warm-up: keep PE busy while input DMAs land ----
            wp = psW.tile([128, 512], F32, tag="w", name="warmps")
            for _ in range(64):
                nc.tensor.matmul(wp[:, 0:128], warm, warm, start=True, stop=True)

            # ---- emission helpers (each is one PE "filler unit") ----
            def qproj_ct(c, ct):
                ps = psW.tile([128, 512], F32, tag="w", name=f"qp{c}_{ct}")
                for o in range(DSUB):
                    nc.tensor.matmul(
                        ps,
                        w_sb[:, o, ct * 128:(ct + 1) * 128],
                        xq_t[:, o, c * QCH:(c + 1) * QCH],
                        start=(o == 0),
                        stop=(o == DSUB - 1),
                    )
                nc.vector.tensor_copy(out=qT[:, ct, c * QCH:(c + 1) * QCH],
                                      in_=ps)

            def xkv_ap(base, w):
                if base == 0:
                    assert w <= 128
                    return xkva[:, :, 0:w]
                return xkvb[:, :, base - 128:base - 128 + w]

            def kproj_ct(wi, ct):
                base, w = kwins[wi]
                src = xkv_ap(base, w)
                ps = psW.tile([128, 512], F32, tag="w", name=f"kp{wi}_{ct}")
                for o in range(DSUB):
                    nc.tensor.matmul(
                        ps[:, :w],
                        w_sb[:, o, 256 + ct * 128:256 + (ct + 1) * 128],
                        src[:, o, :],
                        start=(o == 0),
                        stop=(o == DSUB - 1),
                    )
                nc.vector.tensor_copy(out=kT[:, ct, base:base + w],
                                      in_=ps[:, :w])

            def vproj_kt(kt):
                src = xkv_ap(kt * 128, 128)
                ps = psW.tile([128, 512], F32, tag="w", name=f"vp{kt}")
                pv = ps[:, 0:HL * A]
                for o in range(DSUB):
                    nc.tensor.matmul(
                        pv,
                        src[:, o, :],
                        w_sb[:, o, 512:512 + HL * A],
                        start=(o == 0),
                        stop=(o == DSUB - 1),
                    )
                nc.vector.tensor_copy(
                    out=vx[:, kt, :, 0:A],
                    in_=pv.rearrange("p (h a) -> p h a", a=A),
                )

            prs = {}

            def scores_pair(half, h2, kt):
                """Interleaved T0/T8 score matmuls + 2 exps for a head pair."""
                q0 = half * HALF
                sc_e = psS.tile([128, HALF], F32, tag="sc",
                                name=f"sce{half}_{h2}_{kt}")
                sc_o = psS.tile([128, HALF], F32, tag="sc",
                                name=f"sco{half}_{h2}_{kt}")
                for cc in range(2):
                    qs_ = slice(q0 + cc * QCH, q0 + (cc + 1) * QCH)
                    cs = slice(cc * QCH, (cc + 1) * QCH)
                    nc.tensor.matmul(
                        sc_e[:, cs],
                        kT[0:64, h2, kt * 128:(kt + 1) * 128],
                        qT[0:64, h2, qs_],
                        start=True, stop=True,
                    )
                    nc.tensor.matmul(
                        sc_o[:, cs],
                        kT[64:128, h2, kt * 128:(kt + 1) * 128],
                        qT[64:128, h2, qs_],
                        start=True, stop=True,
                    )
                pr_e = probs.tile([128, HALF], DT_MM, tag="pr",
                                  name=f"pre{half}_{h2}_{kt}")
                pr_o = probs.tile([128, HALF], DT_MM, tag="pr",
                                  name=f"pro{half}_{h2}_{kt}")
                nc.scalar.activation(out=pr_e, in_=sc_e, func=EXP)
                nc.scalar.activation(out=pr_o, in_=sc_o, func=EXP)
                prs[(half, 2 * h2, kt)] = pr_e
                prs[(half, 2 * h2 + 1, kt)] = pr_o

            def pv_unit(h, half, cl, tail=False):
                """PV accumulation for (head, half, chunk-in-half cl).
                Early-drains PSUM to SBUF (pvc) so the bank frees fast.
                Returns a closure finishing the normalize chain."""
                hp, h2 = h % 2, h // 2
                pvt = psW.tile([128, 512], F32, tag="w",
                               name=f"pv{h}_{half}_{cl}")
                pva = pvt[0:A + 1, :]
                for kt in range(KT):
                    nc.tensor.matmul(
                        pva,
                        vx[:, kt, h, :],
                        prs[(half, h, kt)][:, cl * QCH:(cl + 1) * QCH],
                        start=(kt == 0),
                        stop=(kt == KT - 1),
                    )
                pvc = pvcp.tile([65, 512], F32, tag="pvc",
                                name=f"pvc{h}_{half}_{cl}")
                if tail:
                    nc.scalar.copy(out=pvc, in_=pva)
                else:
                    nc.vector.tensor_copy(out=pvc, in_=pva)
                dnd = dramp.tile([1, 512], F32, tag="dnd")
                nc.gpsimd.dma_start(dnd, pvc[64:65, :])
                rBr = norm.tile([64, 512], F32, tag="rBr")
                dnd_b = bass.AP(
                    tensor=dnd.tensor,
                    offset=dnd.offset,
                    ap=[[0, 64], list(dnd.ap[1])],
                )
                nc.gpsimd.dma_start(rBr, dnd_b)

                cslice = slice(half * HALF + cl * QCH,
                               half * HALF + (cl + 1) * QCH)

                def finish():
                    rB = norm.tile([64, 512], F32, tag="rB")
                    nc.vector.reciprocal_approx_fast(rB, rBr)
                    if hp == 0:
                        nc.vector.tensor_tensor(
                            ctxT[0:64, h2, cslice], pvc[0:A, :], rB,
                            mybir.AluOpType.mult,
                        )
                    else:
                        stg = norm.tile([64, 512], DT_MM, tag="stg")
                        nc.vector.tensor_tensor(
                            stg, pvc[0:A, :], rB, mybir.AluOpType.mult
                        )
                        nc.gpsimd.dma_start(ctxT[64:128, h2, cslice], stg)

                return finish

            def outproj_st(st, tail=False):
                """Output projection + bf16 drain + DMA for query tile st.
                In the tail the exp stream is done, so the Scalar engine
                drains one half to halve the DVE serialization."""
                ot = outp.tile([128, D], BF16, tag="ot", name=f"ot{st}")
                for dc in range(2):
                    po = psW.tile([128, 512], F32, tag="w", name=f"po{st}_{dc}")
                    for s2 in range(2):
                        nc.tensor.matmul(
                            po,
                            ctxT[:, s2, st * 128:(st + 1) * 128],
                            wout_sb[:, s2, dc * 512:(dc + 1) * 512],
                            start=(s2 == 0),
                            stop=(s2 == 1),
                        )
                    if tail and dc == 0:
                        nc.scalar.copy(
                            out=ot[:, dc * 512:(dc + 1) * 512], in_=po
                        )
                    else:
                        nc.vector.tensor_copy(
                            out=ot[:, dc * 512:(dc + 1) * 512], in_=po
                        )
                nc.sync.dma_start(out_d.ap()[st * 128:(st + 1) * 128, :], ot)

            # ---- lead-in PE work (minimum needed for first scores) ----
            qproj_ct(0, 0)
            qproj_ct(1, 0)
            kproj_ct(0, 0)

            pending_finish = []

            def run_unit(u):
                # flush one staged normalize-finish before each filler unit so
                # the DVE queue isn't head-of-line blocked on the broadcast DMA
                if pending_finish:
                    pending_finish.pop(0)()
                kind = u[0]
                if kind == "q":
                    qproj_ct(u[1], u[2])
                elif kind == "k":
                    kproj_ct(u[1], u[2])
                elif kind == "v":
                    vproj_kt(u[1])
                elif kind == "pv":
                    pending_finish.append(pv_unit(u[1], u[2], u[3]))
                elif kind == "pvt":
                    pending_finish.append(pv_unit(u[1], u[2], u[3], tail=True))
                elif kind == "st":
                    outproj_st(u[1])
                elif kind == "stt":
                    outproj_st(u[1], tail=True)

            # ---- per-quarter filler slots (emitted AFTER each slot's scores;
            #      every unit must precede, in PE order, the first score matmul
            #      that depends on it) ----
            def build_slots(units, KT):
                slots = [[] for _ in range(KT)]
                for i, u in enumerate(units):
                    slots[min(KT - 1, 1 + i * (KT - 1) // max(1, len(units)))
                          ].append(u)
                return slots

            # quarter (0,0): k-proj ct0 windows must be emitted before the
            # first e-score slot of their keys; ct1 before quarter (0,1);
            # q-ct1 and v-proj spread behind
            q00 = [[] for _ in range(KT)]
            pre_units = {}  # (half,h2,kt) -> units emitted just before its scores
            for wi in range(1, len(kwins)):
                pre_units[(0, 0, kwins[wi][0] // 128)] = [("k", wi, 0)]
                q00[min(3 + wi, KT - 1)].append(("k", wi, 1))
            q00[min(1, KT - 1)].append(("k", 0, 1))
            nv0 = max(0, KT - 2)  # v-proj kts kept in quarter (0,0)
            rest = [("q", 0, 1), ("q", 1, 1)] + [("v", kt) for kt in range(nv0)]
            for i, u in enumerate(rest):
                q00[min(KT - 1, 2 + i * (KT - 3) // max(1, len(rest)))].append(u)

            quarter_slots = {
                (0, 0): q00,
                (0, 1): build_slots(
                    [("v", kt) for kt in range(nv0, KT)] +
                    [("q", 2, 0), ("q", 3, 0),
                     ("pv", 0, 0, 0), ("pv", 1, 0, 0)], KT),
                (1, 0): build_slots(
                    [("pv", 0, 0, 1), ("pv", 1, 0, 1),
                     ("q", 2, 1), ("q", 3, 1),
                     ("pv", 2, 0, 0), ("pv", 3, 0, 0), ("st", 0),
                     ("pv", 2, 0, 1), ("pv", 3, 0, 1), ("st", 1)], KT),
                (1, 1): build_slots(
                    [("pv", 0, 1, 0), ("pv", 1, 1, 0),
                     ("pv", 0, 1, 1), ("pv", 1, 1, 1),
                     ("st", 2), ("st", 3), ("st", 4), ("st", 5),
                     ("st", 6), ("st", 7)], KT),
            }

            # scores are emitted one slot AHEAD of fillers so the PE always
            # has the next pair of score matmuls queued while exp streams
            slot_seq = [(half, h2, kt)
                        for half in range(2) for h2 in range(2)
                        for kt in range(KT)]
            scores_pair(*slot_seq[0])
            for i, (half, h2, kt) in enumerate(slot_seq):
                if i + 1 < len(slot_seq):
                    nxt = slot_seq[i + 1]
                    for u in pre_units.get(nxt, []):
                        run_unit(u)
                    scores_pair(*nxt)
                for u in quarter_slots[(half, h2)][kt]:
                    run_unit(u)

            # ---- tail: last pair's PV + second-half out-proj ----
            for u in [("pvt", 2, 1, 0), ("pvt", 3, 1, 0),
                      ("pvt", 2, 1, 1), ("pvt", 3, 1, 1),
                      ("stt", 8), ("stt", 9), ("stt", 10), ("stt", 11),
                      ("stt", 12), ("stt", 13), ("stt", 14), ("stt", 15)]:
                run_unit(u)
            while pending_finish:
                pending_finish.pop(0)()

    return nc


_PROGRAM_CACHE: dict[int, bass.Bass] = {}


def _get_program(C: int) -> bass.Bass:
    if C not in _PROGRAM_CACHE:
        nc = build_program(C)
        nc.finalize()
        _PROGRAM_CACHE[C] = nc
    return _PROGRAM_CACHE[C]


def _ceil128(n: int) -> int:
    return max(128, (n + 127) // 128 * 128)


def prepare_in_maps(qs, mask, Wqkv, Wout):
    """Shard FULL inputs into 8 per-core input maps. Returns (in_maps, C)."""
    import ml_dtypes

    np_mm = ml_dtypes.bfloat16
    qs = np.ascontiguousarray(qs, dtype=np.float32)
    mask = np.asarray(mask)
    Wqkv = np.ascontiguousarray(Wqkv, dtype=np.float32)
    Wout = np.ascontiguousarray(Wout, dtype=np.float32)

    nvalid = [int(np.count_nonzero(mask[b])) for b in range(B)]
    if min(nvalid) == 0:
        C = S  # degenerate masks: run dense
    else:
        C = min(S, _ceil128(max(nvalid)))
    compact = C < S

    xq, xkv, mc = [], [], []
    for b in range(B):
        xq.append(np.ascontiguousarray(qs[b].T.astype(np_mm)))
        if compact:
            idx = np.nonzero(mask[b] != 0)[0]
            xs = np.zeros((C, D), dtype=np.float32)
            xs[: len(idx)] = qs[b][idx]
            mcol = np.zeros(C, dtype=np.float32)
            mcol[: len(idx)] = 1.0
        else:
            mcol = (mask[b] != 0).astype(np.float32)
            xs = qs[b] * mcol[:, None]
        xkv.append(np.ascontiguousarray(xs.T.astype(np_mm)))
        mc.append(mcol.astype(np_mm))

    in_maps = []
    for b in range(B):
        for g in range(HG):
            h0 = g * HL
            wq = Wqkv[:, (0 * H + h0) * A:(0 * H + h0 + HL) * A] * (
                1.0 / np.sqrt(np.float32(A))
            )
            wk = Wqkv[:, (1 * H + h0) * A:(1 * H + h0 + HL) * A]
            wv = Wqkv[:, (2 * H + h0) * A:(2 * H + h0 + HL) * A]
            wcat = np.concatenate([wq, wk, wv], axis=1, dtype=np.float32)
            # device layout [128, DSUB, 768]: row o*128+p -> [p, o, :]
            wqkv_s = np.ascontiguousarray(
                wcat.reshape(DSUB, 128, 3 * HL * A).transpose(1, 0, 2)
                .astype(np_mm)
            )
            # device layout [128, 2, D]: row s2*128+p -> [p, s2, :]
            wout_s = np.ascontiguousarray(
                Wout[h0 * A:(h0 + HL) * A, :]
                .reshape(2, 128, D).transpose(1, 0, 2).astype(np_mm)
            )
            in_maps.append(
                {
                    "xq": xq[b],
                    "xkv": xkv[b],
                    "wqkv": wqkv_s,
                    "wout": wout_s,
                    "mcol": mc[b],
                }
            )
    return in_maps, C


def gather_output(results, bout):
    """Sum the 4 head-group partials per batch (f32) and add bout."""
    out = np.empty((B, S, D), dtype=np.float32)
    for b in range(B):
        acc = results[b * HG]["out"].astype(np.float32)
        for g in range(1, HG):
            acc = acc + results[b * HG + g]["out"].astype(np.float32)
        out[b] = acc + bout.astype(np.float32)[None, :]
    return out


def _ensure_ntff_hook():
    """Inject antenv.axon_hooks (missing on this image) so trace=True works."""
    import sys
    import types

    try:
        from antenv import axon_hooks  # noqa: F401
        return
    except ImportError:
        pass
    mod = types.ModuleType("antenv.axon_hooks")
    _h = [None]
    mod.set_axon_ntff_profile_hook = lambda h: _h.__setitem__(0, h)
    mod.get_axon_ntff_profile_hook = lambda: _h[0]
    sys.modules["antenv.axon_hooks"] = mod
    import antenv

    antenv.axon_hooks = mod
    try:
        from trn_agent_boot.trn_boot import _ntff_profile_via_ctypes

        mod.set_axon_ntff_profile_hook(
            _ntff_profile_via_ctypes("/opt/axon/libaxon_pjrt.so")
        )
    except Exception:
        pass


def run(qs, mask, Wqkv, Wout, bout, trace=False):
    if trace:
        _ensure_ntff_hook()
    in_maps, C = prepare_in_maps(qs, mask, Wqkv, Wout)
    nc = _get_program(C)
    res = run_bass_kernel_spmd(
        nc, in_maps, core_ids=list(range(B * HG)), trace=trace
    )
    return gather_output(res.results, np.asarray(bout)), res


def kernel(qs, mask, Wqkv, Wout, bout):
    return run(qs, mask, Wqkv, Wout, bout, trace=False)[0]


# revision 22
# speedup vs baseline: 1.2117x; 1.0073x over previous
"""Multi-head attention (B=2, S=2048, D=1024, H=16, A=64) on 8 TRN2 NeuronCores.

Sharding: core c = b*4 + g handles batch b and head-group g (4 heads).
Tensor-parallel over heads; the all-reduce over head groups is host-side
during the gather (sum of 4 bf16 partials per batch, f32 accumulate).

Key structure (v2 — exp-stream-centric schedule):
 - The Scalar (ACT) engine's exp stream is the critical path (~9.4M exps/core).
   The schedule starts that stream as early as possible and keeps it gapless:
   scores for one head pair land in two alternating PSUM tiles (sc_e/sc_o)
   that exp drains back-to-back while the PE refills the other; everything
   else (projections, PV, out-proj) is PE filler emitted between score
   matmuls in dependency-safe program order.
 - Masking is folded into the data, not the exp: invalid/padded key columns
   of xkv are zeroed host-side (k=0 -> score 0 -> exp 1 -> v=0 contributes
   nothing) and the softmax denominator comes from a 0/1 mask column
   appended to V, so masked keys count 0 in the denominator too.
 - Scores for the two heads of a pair are emitted interleaved (rows 0:64 via
   PE tile T0, rows 64:128 via T8) so the row-tiled PE can overlap them.
 - ~140 dummy warm-up matmuls at t=0 keep the PE busy while input DMAs land,
   flipping the HAM clock gate to 2.4 GHz before real work starts.
 - Output projection streams per 128-query tile into bf16 DRAM as soon as
   each half's context is normalized; host gather sums partials in f32.
"""

import numpy as np

import concourse.bass as bass
import concourse.bacc as bacc
import concourse.mybir as mybir
import concourse.tile as tile
from concourse.bass_utils import run_bass_kernel_spmd

F32 = mybir.dt.float32
BF16 = mybir.dt.bfloat16
DT_MM = BF16
EXP = mybir.ActivationFunctionType.Exp

B = 2
S = 2048
D = 1024
H = 16
A = 64
HG = 4           # head groups (cores per batch)
HL = H // HG     # heads per core = 4
DSUB = D // 128  # 8
QCH = 512        # matmul qi chunk
HALF = 1024      # qi half processed per quarter


def build_program(C: int) -> bass.Bass:
    """Per-core SPMD program for key-capacity C (multiple of 128)."""
    KT = C // 128
    # kv windows: first is 128 keys so scores can start early
    kwins = [(0, 128)]
    pos = 128
    while pos < C:
        w = min(QCH, C - pos)
        kwins.append((pos, w))
        pos += w
    n_pr_bufs = 24 if KT > 10 else min(44, 4 * KT + 4)

    nc = bacc.Bacc("TRN2", target_bir_lowering=False, name=f"mha2_c{C}")
    xq_d = nc.dram_tensor("xq", [D, S], DT_MM, kind="ExternalInput")
    xkv_d = nc.dram_tensor("xkv", [D, C], DT_MM, kind="ExternalInput")
    wqkv_d = nc.dram_tensor("wqkv", [128, DSUB, 3 * HL * A], DT_MM,
                            kind="ExternalInput")
    wout_d = nc.dram_tensor("wout", [128, 2, D], DT_MM, kind="ExternalInput")
    mcol_d = nc.dram_tensor("mcol", [C], DT_MM, kind="ExternalInput")
    out_d = nc.dram_tensor("out", [S, D], BF16, kind="ExternalOutput")

    with tile.TileContext(nc) as tc:
        with (
            tc.tile_pool(name="const", bufs=1) as const,
            tc.tile_pool(name="probs", bufs=n_pr_bufs) as probs,
            tc.tile_pool(name="norm", bufs=3) as norm,
            tc.tile_pool(name="pvcp", bufs=5) as pvcp,
            tc.tile_pool(name="outp", bufs=3) as outp,
            tc.tile_pool(name="psS", bufs=3, space="PSUM") as psS,
            tc.tile_pool(name="psW", bufs=2, space="PSUM") as psW,
        ):
            # ---- persistent SBUF residents ----
            w_sb = const.tile([128, DSUB, 3 * HL * A], DT_MM)
            wout_sb = const.tile([128, 2, D], DT_MM)
            qT = const.tile([128, 2, S], DT_MM)        # [hp*64+a, h2, qi]
            kT = const.tile([128, 2, C], DT_MM)        # [hp*64+a, h2, ki]
            vx = const.tile([128, KT, HL, A + 1], DT_MM)  # [ki%128, kt, h, a|mask]
            ctxT = const.tile([128, 2, S], DT_MM)      # [hp*64+a, h2, qi]
            warm = const.tile([128, 128], DT_MM)
            ones_z = const.tile([96, 64], F32)
            mc_sb = const.tile([128, KT], DT_MM)
            xq_t = const.tile([128, DSUB, S], DT_MM)
            xkva = const.tile([128, DSUB, 128], DT_MM)
            xkvb = (const.tile([128, DSUB, C - 128], DT_MM, name="xkvb")
                    if C > 128 else None)

            nc.vector.memset(warm, 0.0)
            nc.vector.memset(ones_z, 0.0)
            nc.vector.memset(ones_z[64:65, :], 1.0)

            xq_r = xq_d.ap().rearrange("(o p) s -> p o s", p=128)
            xkv_r = xkv_d.ap().rearrange("(o p) s -> p o s", p=128)
            mcol_r = mcol_d.ap().rearrange("(t p) -> p t", p=128)

            # ---- DMA issue order (earliest-needed first); x transfers are
            #      per-(o, s-range) so each DRAM descriptor is a 2KB+ run ----
            nc.sync.dma_start(w_sb, wqkv_d.ap())
            for o in range(DSUB):
                nc.sync.dma_start(xq_t[:, o, 0:1024], xq_r[:, o, 0:1024])
            nc.sync.dma_start(xkva, xkv_r[:, :, 0:128])
            xsplit = min(640, C)
            for o in range(DSUB):
                nc.sync.dma_start(xkvb[:, o, 0:xsplit - 128],
                                  xkv_r[:, o, 128:xsplit])
            if C > 640:
                for o in range(DSUB):
                    nc.sync.dma_start(xkvb[:, o, 512:C - 128],
                                      xkv_r[:, o, 640:C])
            for o in range(DSUB):
                nc.sync.dma_start(xq_t[:, o, 1024:2048], xq_r[:, o, 1024:2048])
            nc.sync.dma_start(mc_sb, mcol_r)
            nc.sync.dma_start(wout_sb, wout_d.ap())
            # mask column into vx via DVE (element-exact strided writes; a
            # strided 2-byte DMA scatter here raced with the v-proj drains)
            for h in range(HL):
                nc.vector.tensor_copy(out=vx[:, :, h, A], in_=mc_sb)

            # ---- HAM warm-up: keep PE busy while input DMAs land ----
            wp = psW.tile([128, 512], F32, tag="w", name="warmps")
            for _ in range(64):
                nc.tensor.matmul(wp[:, 0:128], warm, warm, start=True, stop=True)

            # ---- emission helpers (each is one PE "filler unit") ----
            def qproj_ct(c, ct):
                ps = psW.tile([128, 512], F32, tag="w", name=f"qp{c}_{ct}")
                for o in range(DSUB):
                    nc.tensor.matmul(
                        ps,
                        w_sb[:, o, ct * 128:(ct + 1) * 128],
                        xq_t[:, o, c * QCH:(c + 1) * QCH],
                        start=(o == 0),
                        stop=(o == DSUB - 1),
                    )
                nc.vector.tensor_copy(out=qT[:, ct, c * QCH:(c + 1) * QCH],
                                      in_=ps)

            def xkv_ap(base, w):
                if base == 0:
                    assert w <= 128
                    return xkva[:, :, 0:w]
                return xkvb[:, :, base - 128:base - 128 + w]

            def kproj_ct(wi, ct):
                base, w = kwins[wi]
                src = xkv_ap(base, w)
                ps = psW.tile([128, 512], F32, tag="w", name=f"kp{wi}_{ct}")
                for o in range(DSUB):
                    nc.tensor.matmul(
                        ps[:, :w],
                        w_sb[:, o, 256 + ct * 128:256 + (ct + 1) * 128],
                        src[:, o, :],
                        start=(o == 0),
                        stop=(o == DSUB - 1),
                    )
                nc.vector.tensor_copy(out=kT[:, ct, base:base + w],
                                      in_=ps[:, :w])

            def vproj_kt(kt):
                src = xkv_ap(kt * 128, 128)
                ps = psW.tile([128, 512], F32, tag="w", name=f"vp{kt}")
                pv = ps[:, 0:HL * A]
                for o in range(DSUB):
                    nc.tensor.matmul(
                        pv,
                        src[:, o, :],
                        w_sb[:, o, 512:512 + HL * A],
                        start=(o == 0),
                        stop=(o == DSUB - 1),
                    )
                nc.vector.tensor_copy(
                    out=vx[:, kt, :, 0:A],
                    in_=pv.rearrange("p (h a) -> p h a", a=A),
                )

            prs = {}

            def scores_pair(half, h2, kt):
                """Interleaved T0/T8 score matmuls + 2 exps for a head pair."""
                q0 = half * HALF
                sc_e = psS.tile([128, HALF], F32, tag="sc",
                                name=f"sce{half}_{h2}_{kt}")
                sc_o = psS.tile([128, HALF], F32, tag="sc",
                                name=f"sco{half}_{h2}_{kt}")
                for cc in range(2):
                    qs_ = slice(q0 + cc * QCH, q0 + (cc + 1) * QCH)
                    cs = slice(cc * QCH, (cc + 1) * QCH)
                    nc.tensor.matmul(
                        sc_e[:, cs],
                        kT[0:64, h2, kt * 128:(kt + 1) * 128],
                        qT[0:64, h2, qs_],
                        start=True, stop=True,
                    )
                    nc.tensor.matmul(
                        sc_o[:, cs],
                        kT[64:128, h2, kt * 128:(kt + 1) * 128],
                        qT[64:128, h2, qs_],
                        start=True, stop=True,
                    )
                pr_e = probs.tile([128, HALF], DT_MM, tag="pr",
                                  name=f"pre{half}_{h2}_{kt}")
                pr_o = probs.tile([128, HALF], DT_MM, tag="pr",
                                  name=f"pro{half}_{h2}_{kt}")
                prs[(half, 2 * h2, kt)] = pr_e
                prs[(half, 2 * h2 + 1, kt)] = pr_o
                return (sc_e, sc_o, pr_e, pr_o)

            def pv_unit(h, half, cl, tail=False):
                """PV accumulation for (head, half, chunk-in-half cl).
                Early-drains PSUM to SBUF (pvc) so the bank frees fast.
                Returns a closure finishing the normalize chain."""
                hp, h2 = h % 2, h // 2
                pvt = psW.tile([128, 512], F32, tag="w",
                               name=f"pv{h}_{half}_{cl}")
                pva = pvt[0:A + 1, :]
                for kt in range(KT):
                    nc.tensor.matmul(
                        pva,
                        vx[:, kt, h, :],
                        prs[(half, h, kt)][:, cl * QCH:(cl + 1) * QCH],
                        start=(kt == 0),
                        stop=(kt == KT - 1),
                    )
                pvc = pvcp.tile([96, 512], F32, tag="pvc",
                                name=f"pvc{h}_{half}_{cl}")
                if tail:
                    nc.scalar.copy(out=pvc, in_=pvt[0:96, :])
                else:
                    nc.vector.tensor_copy(out=pvc, in_=pvt[0:96, :])
                rb_ps = psW.tile([128, 512], F32, tag="w",
                                 name=f"rb{h}_{half}_{cl}")
                nc.tensor.matmul(rb_ps[0:64, :], ones_z[64:96, :],
                                 pvc[64:96, :], start=True, stop=True)

                cslice = slice(half * HALF + cl * QCH,
                               half * HALF + (cl + 1) * QCH)

                def finish():
                    rB = norm.tile([64, 512], F32, tag="rB")
                    nc.vector.reciprocal_approx_fast(rB, rb_ps[0:64, :])
                    if hp == 0:
                        nc.vector.tensor_tensor(
                            ctxT[0:64, h2, cslice], pvc[0:A, :], rB,
                            mybir.AluOpType.mult,
                        )
                    else:
                        stg = norm.tile([64, 512], DT_MM, tag="stg")
                        nc.vector.tensor_tensor(
                            stg, pvc[0:A, :], rB, mybir.AluOpType.mult
                        )
                        nc.gpsimd.dma_start(ctxT[64:128, h2, cslice], stg)

                return finish

            def outproj_st(st, tail=False):
                """Output projection + bf16 drain + DMA for query tile st.
                In the tail the exp stream is done, so the Scalar engine
                drains one half to halve the DVE serialization."""
                ot = outp.tile([128, D], BF16, tag="ot", name=f"ot{st}")
                for dc in range(2):
                    po = psW.tile([128, 512], F32, tag="w", name=f"po{st}_{dc}")
                    for s2 in range(2):
                        nc.tensor.matmul(
                            po,
                            ctxT[:, s2, st * 128:(st + 1) * 128],
                            wout_sb[:, s2, dc * 512:(dc + 1) * 512],
                            start=(s2 == 0),
                            stop=(s2 == 1),
                        )
                    if tail and dc == 0:
                        nc.scalar.copy(
                            out=ot[:, dc * 512:(dc + 1) * 512], in_=po
                        )
                    else:
                        nc.vector.tensor_copy(
                            out=ot[:, dc * 512:(dc + 1) * 512], in_=po
                        )
                nc.sync.dma_start(out_d.ap()[st * 128:(st + 1) * 128, :], ot)

            # ---- lead-in PE work (minimum needed for first scores) ----
            qproj_ct(0, 0)
            qproj_ct(1, 0)
            kproj_ct(0, 0)

            pending_finish = []

            def run_unit(u):
                # flush one staged normalize-finish before each filler unit so
                # the DVE queue isn't head-of-line blocked on the broadcast DMA
                if pending_finish:
                    pending_finish.pop(0)()
                kind = u[0]
                if kind == "q":
                    qproj_ct(u[1], u[2])
                elif kind == "k":
                    kproj_ct(u[1], u[2])
                elif kind == "v":
                    vproj_kt(u[1])
                elif kind == "pv":
                    pending_finish.append(pv_unit(u[1], u[2], u[3]))
                elif kind == "pvt":
                    pending_finish.append(pv_unit(u[1], u[2], u[3], tail=True))
                elif kind == "st":
                    outproj_st(u[1])
                elif kind == "stt":
                    outproj_st(u[1], tail=True)

            # ---- per-quarter filler slots (emitted AFTER each slot's scores;
            #      every unit must precede, in PE order, the first score matmul
            #      that depends on it) ----
            def build_slots(units, KT):
                slots = [[] for _ in range(KT)]
                for i, u in enumerate(units):
                    slots[min(KT - 1, 1 + i * (KT - 1) // max(1, len(units)))
                          ].append(u)
                return slots

            # quarter (0,0): k-proj ct0 windows must be emitted before the
            # first e-score slot of their keys; ct1 before quarter (0,1);
            # q-ct1 and v-proj spread behind
            q00 = [[] for _ in range(KT)]
            pre_units = {}  # (half,h2,kt) -> units emitted just before its scores
            for wi in range(1, len(kwins)):
                pre_units[(0, 0, kwins[wi][0] // 128)] = [("k", wi, 0)]
                q00[min(3 + wi, KT - 1)].append(("k", wi, 1))
            q00[min(1, KT - 1)].append(("k", 0, 1))
            nv0 = max(0, KT - 2)  # v-proj kts kept in quarter (0,0)
            rest = [("q", 0, 1), ("q", 1, 1)] + [("v", kt) for kt in range(nv0)]
            for i, u in enumerate(rest):
                q00[min(KT - 1, 2 + i * (KT - 3) // max(1, len(rest)))].append(u)

            quarter_slots = {
                (0, 0): q00,
                (0, 1): build_slots(
                    [("v", kt) for kt in range(nv0, KT)] +
                    [("q", 2, 0), ("q", 3, 0),
                     ("pv", 0, 0, 0), ("pv", 1, 0, 0)], KT),
                (1, 0): build_slots(
                    [("pv", 0, 0, 1), ("pv", 1, 0, 1),
                     ("pv", 2, 0, 0), ("pv", 3, 0, 0),
                     ("pv", 2, 0, 1), ("pv", 3, 0, 1),
                     ("q", 2, 1), ("q", 3, 1), ("st", 0), ("st", 1)], KT),
                (1, 1): build_slots(
                    [("pv", 0, 1, 0), ("pv", 1, 1, 0),
                     ("pv", 0, 1, 1), ("pv", 1, 1, 1),
                     ("st", 2), ("st", 3), ("st", 4), ("st", 5),
                     ("st", 6), ("st", 7)], KT),
            }

            # scores are emitted one slot AHEAD of fillers so the PE always
            # has the next pair of score matmuls queued while exp streams
            slot_seq = [(half, h2, kt)
                        for half in range(2) for h2 in range(2)
                        for kt in range(KT)]
            # exps for slot i are emitted AFTER slot i+1's score matmuls so
            # the concurrently-executing row-tile pair has fully drained to
            # PSUM before the Scalar engine reads it
            def emit_exps(p):
                sc_e, sc_o, pr_e, pr_o = p
                nc.scalar.activation(out=pr_e, in_=sc_e, func=EXP)
                nc.scalar.activation(out=pr_o, in_=sc_o, func=EXP)

            cur = scores_pair(*slot_seq[0])
            for i, (half, h2, kt) in enumerate(slot_seq):
                nxt_p = None
                if i + 1 < len(slot_seq):
                    nxt = slot_seq[i + 1]
                    for u in pre_units.get(nxt, []):
                        run_unit(u)
                    nxt_p = scores_pair(*nxt)
                emit_exps(cur)
                cur = nxt_p
                for u in quarter_slots[(half, h2)][kt]:
                    run_unit(u)

            # ---- tail: last pair's PV + second-half out-proj ----
            for u in [("pvt", 2, 1, 0), ("pvt", 3, 1, 0),
                      ("pvt", 2, 1, 1), ("pvt", 3, 1, 1),
                      ("stt", 8), ("stt", 9), ("stt", 10), ("stt", 11),
                      ("stt", 12), ("stt", 13), ("stt", 14), ("stt", 15)]:
                run_unit(u)
            while pending_finish:
                pending_finish.pop(0)()

    return nc


_PROGRAM_CACHE: dict[int, bass.Bass] = {}


def _get_program(C: int) -> bass.Bass:
    if C not in _PROGRAM_CACHE:
        nc = build_program(C)
        nc.finalize()
        _PROGRAM_CACHE[C] = nc
    return _PROGRAM_CACHE[C]


def _ceil128(n: int) -> int:
    return max(128, (n + 127) // 128 * 128)


def prepare_in_maps(qs, mask, Wqkv, Wout):
    """Shard FULL inputs into 8 per-core input maps. Returns (in_maps, C)."""
    import ml_dtypes

    np_mm = ml_dtypes.bfloat16
    qs = np.ascontiguousarray(qs, dtype=np.float32)
    mask = np.asarray(mask)
    Wqkv = np.ascontiguousarray(Wqkv, dtype=np.float32)
    Wout = np.ascontiguousarray(Wout, dtype=np.float32)

    nvalid = [int(np.count_nonzero(mask[b])) for b in range(B)]
    if min(nvalid) == 0:
        C = S  # degenerate masks: run dense
    else:
        C = min(S, _ceil128(max(nvalid)))
    compact = C < S

    xq, xkv, mc = [], [], []
    for b in range(B):
        xq.append(np.ascontiguousarray(qs[b].T.astype(np_mm)))
        if compact:
            idx = np.nonzero(mask[b] != 0)[0]
            xs = np.zeros((C, D), dtype=np.float32)
            xs[: len(idx)] = qs[b][idx]
            mcol = np.zeros(C, dtype=np.float32)
            mcol[: len(idx)] = 1.0
        else:
            mcol = (mask[b] != 0).astype(np.float32)
            xs = qs[b] * mcol[:, None]
        xkv.append(np.ascontiguousarray(xs.T.astype(np_mm)))
        mc.append(mcol.astype(np_mm))

    in_maps = []
    for b in range(B):
        for g in range(HG):
            h0 = g * HL
            wq = Wqkv[:, (0 * H + h0) * A:(0 * H + h0 + HL) * A] * (
                1.0 / np.sqrt(np.float32(A))
            )
            wk = Wqkv[:, (1 * H + h0) * A:(1 * H + h0 + HL) * A]
            wv = Wqkv[:, (2 * H + h0) * A:(2 * H + h0 + HL) * A]
            wcat = np.concatenate([wq, wk, wv], axis=1, dtype=np.float32)
            # device layout [128, DSUB, 768]: row o*128+p -> [p, o, :]
            wqkv_s = np.ascontiguousarray(
                wcat.reshape(DSUB, 128, 3 * HL * A).transpose(1, 0, 2)
                .astype(np_mm)
            )
            # device layout [128, 2, D]: row s2*128+p -> [p, s2, :]
            wout_s = np.ascontiguousarray(
                Wout[h0 * A:(h0 + HL) * A, :]
                .reshape(2, 128, D).transpose(1, 0, 2).astype(np_mm)
            )
            in_maps.append(
                {
                    "xq": xq[b],
                    "xkv": xkv[b],
                    "wqkv": wqkv_s,
                    "wout": wout_s,
                    "mcol": mc[b],
                }
            )
    return in_maps, C


def gather_output(results, bout):
    """Sum the 4 head-group partials per batch (f32) and add bout."""
    out = np.empty((B, S, D), dtype=np.float32)
    for b in range(B):
        acc = results[b * HG]["out"].astype(np.float32)
        for g in range(1, HG):
            acc = acc + results[b * HG + g]["out"].astype(np.float32)
        out[b] = acc + bout.astype(np.float32)[None, :]
    return out


def _ensure_ntff_hook():
    """Inject antenv.axon_hooks (missing on this image) so trace=True works."""
    import sys
    import types

    try:
        from antenv import axon_hooks  # noqa: F401
        return
    except ImportError:
        pass
    mod = types.ModuleType("antenv.axon_hooks")
    _h = [None]
    mod.set_axon_ntff_profile_hook = lambda h: _h.__setitem__(0, h)
    mod.get_axon_ntff_profile_hook = lambda: _h[0]
    sys.modules["antenv.axon_hooks"] = mod
    import antenv

    antenv.axon_hooks = mod
    try:
        from trn_agent_boot.trn_boot import _ntff_profile_via_ctypes

        mod.set_axon_ntff_profile_hook(
            _ntff_profile_via_ctypes("/opt/axon/libaxon_pjrt.so")
        )
    except Exception:
        pass


def run(qs, mask, Wqkv, Wout, bout, trace=False):
    if trace:
        _ensure_ntff_hook()
    in_maps, C = prepare_in_maps(qs, mask, Wqkv, Wout)
    nc = _get_program(C)
    res = run_bass_kernel_spmd(
        nc, in_maps, core_ids=list(range(B * HG)), trace=trace
    )
    return gather_output(res.results, np.asarray(bout)), res


def kernel(qs, mask, Wqkv, Wout, bout):
    return run(qs, mask, Wqkv, Wout, bout, trace=False)[0]
